# revision 1
# baseline (speedup 1.0000x reference)
"""Trainium2 Bass kernel: causal MultiHeadAttention with RoPE.

B=1, S=4096, D=768, H=12 heads, dk=64, fp32 I/O. 8 NeuronCores, SPMD.

Sharding: snake-interleaved query tiles. Core c owns the two 256-row query
tiles {c, 8+c} (of 16), which balances causal attention work exactly. Every
core redundantly computes the full K and V projections (cheap vs. any
collective), computes flash-style attention for its 512 query rows over all
12 heads, applies the output projection for those rows, and writes its
[512, 768] slice. The host scatters slices into the full output.

Device-side layouts (all produced by host-side prep, no device transposes):
  - xt:  X^T [768, 4096] bf16, k-tile columns permuted per-core (sigma) so
         the attention loop's k-iteration order is static & uniform.
  - wq/wk/wv/wo: W^T [768, 768] bf16 (matmul contraction on partitions).
  - cos/sin tables for RoPE in the [d, s] layout (pair-swap via
    stream_shuffle; sign baked into the sin table).
  - causal handling: two static triangle masks for the diagonal k-tiles
    (always iterations 0,1 of each slot) + per-pair exp bias (-100 kills
    padded tiles) supplied as data, keeping one identical program per core.
  - softmax denominators via a ones-column appended to V (row 64 of the
    PV accumulator); per-head normalization with DVE reciprocal + GpSimd
    partition_broadcast; normalized attention lands directly in the
    o_proj stationary layout.
"""

import sys

if "/opt/trn_rl_repo" not in sys.path:
    sys.path.insert(0, "/opt/trn_rl_repo")

import numpy as np
import ml_dtypes

D_MODEL = 768
H = 12
DK = 64
S = 4096
THETA = 10000.0
MAX_SEQ_LEN = 4096
N_CORES = 8
QT = 256            # query rows per slot
N_KT = S // 128     # 32 k-tiles of 128
EB = D_MODEL // 128  # 6 e/d blocks of 128
N_CH = S // 512     # 8 projection chunks of 512
VW = H * 65         # V_aug row width per s-tile (12 heads x (64+ones))

BF16 = ml_dtypes.bfloat16

# Iteration -> storage-slot maps (identical on every core; per-core variation
# is entirely in the data: sigma-permuted xt/cos/sin, bias tables).
IT0_MAP = [0, 1] + list(range(4, 18))            # slot0: 16 iterations
IT1_MAP = [2, 3, 0, 1] + list(range(4, 32))      # slot1: 32 iterations


def _sigma(c):
    """Storage permutation: which k-tile sits in storage slot i for core c."""
    specials = [2 * c, 2 * c + 1, 2 * c + 16, 2 * c + 17]
    rest = [t for t in range(N_KT) if t not in specials]
    return specials + rest


def _bias_cols(c):
    """Per-exp-pair bias: 0.0 keeps the pair of k-tiles, -100 kills it."""
    sig = _sigma(c)
    cols = []
    # slot0 (q-tile T=c, live k-tiles [0, 2c+2)): 8 pairs
    for p in range(8):
        if p == 0:
            cols.append(0.0)  # diagonal pair, masked
        else:
            tid = sig[IT0_MAP[2 * p]]
            cols.append(0.0 if tid <= 2 * c - 1 else -100.0)
    # slot1 (q-tile T=8+c, live k-tiles [0, 2c+18)): 16 pairs
    for p in range(16):
        if p == 0:
            cols.append(0.0)  # diagonal pair
        elif p == 1:
            cols.append(0.0)  # storage 0,1 = tiles 2c,2c+1, always live
        else:
            tid = sig[IT1_MAP[2 * p]]
            cols.append(0.0 if tid < 2 * c + 16 else -100.0)
    return np.asarray(cols, np.float32)


def build_program():
    import concourse.mybir as mybir
    import concourse.tile as tile
    from concourse import bacc, library_config

    f32 = mybir.dt.float32
    bf16 = mybir.dt.bfloat16
    Exp = mybir.ActivationFunctionType.Exp
    Copy = mybir.ActivationFunctionType.Copy

    nc = bacc.Bacc(
        "TRN2",
        target_bir_lowering=False,
        debug=False,
        enable_asserts=True,
        num_devices=N_CORES,
    )

    xt_d = nc.dram_tensor("xt", [D_MODEL, S], bf16, kind="ExternalInput")
    xtq_d = nc.dram_tensor("xtq", [D_MODEL, 2 * QT], bf16, kind="ExternalInput")
    w_d = {
        n: nc.dram_tensor(n, [D_MODEL, D_MODEL], bf16, kind="ExternalInput")
        for n in ("wq", "wk", "wv", "wo")
    }
    cosk_d = nc.dram_tensor("cosk", [128, S], bf16, kind="ExternalInput")
    sink_d = nc.dram_tensor("sink", [128, S], bf16, kind="ExternalInput")
    cosq_d = nc.dram_tensor("cosq", [128, 2 * QT], bf16, kind="ExternalInput")
    sinq_d = nc.dram_tensor("sinq", [128, 2 * QT], bf16, kind="ExternalInput")
    mask_d = nc.dram_tensor("maskab", [128, 512], bf16, kind="ExternalInput")
    bias_d = nc.dram_tensor("biasp", [128, 24], f32, kind="ExternalInput")
    out_d = nc.dram_tensor("out", [2 * QT, D_MODEL], f32, kind="ExternalOutput")

    PAIRSWAP = [i ^ 1 for i in range(32)]

    with tile.TileContext(nc) as tc:
        with (
            tc.tile_pool(name="const", bufs=1) as cpool,
            tc.tile_pool(name="wp", bufs=2) as wpool,
            tc.tile_pool(name="cs", bufs=2) as cspool,
            tc.tile_pool(name="xtp", bufs=2) as xtpool,
            tc.tile_pool(name="rope", bufs=3) as rpool,
            tc.tile_pool(name="expp", bufs=4) as epool,
            tc.tile_pool(name="norm", bufs=2) as npool,
            tc.tile_pool(name="outp", bufs=2) as opool,
            tc.tile_pool(name="ps_proj", bufs=2, space="PSUM") as psb,
            tc.tile_pool(name="ps_sc", bufs=3, space="PSUM") as pssc,
            tc.tile_pool(name="ps_small", bufs=1, space="PSUM") as pss,
            tc.tile_pool(name="ps_pv", bufs=2, space="PSUM") as psv,
        ):
            nc.gpsimd.load_library(library_config.attn)

            # ---- persistent tensors (allocated once) ----
            def load_w(n):
                t = wpool.tile([128, EB * D_MODEL], bf16, tag="w")
                for eb in range(EB):
                    nc.sync.dma_start(
                        out=t[:, eb * D_MODEL:(eb + 1) * D_MODEL],
                        in_=w_d[n][eb * 128:(eb + 1) * 128, :],
                    )
                return t
            cosq = cpool.tile([128, 2 * QT], bf16, tag="cosq")
            nc.sync.dma_start(out=cosq[:], in_=cosq_d[:])
            sinq = cpool.tile([128, 2 * QT], bf16, tag="sinq")
            nc.sync.dma_start(out=sinq[:], in_=sinq_d[:])
            maskab = cpool.tile([128, 512], bf16, tag="maskab")
            nc.sync.dma_start(out=maskab[:], in_=mask_d[:])
            biasp = cpool.tile([128, 24], f32, tag="biasp")
            nc.sync.dma_start(out=biasp[:], in_=bias_d[:])
            xtq = cpool.tile([128, EB * 2 * QT], bf16, tag="xtq")
            for eb in range(EB):
                nc.sync.dma_start(
                    out=xtq[:, eb * 2 * QT:(eb + 1) * 2 * QT],
                    in_=xtq_d[eb * 128:(eb + 1) * 128, :],
                )

            # K^T and V_aug split per chunk so attention iterations only
            # depend on the chunk that produced their k-tiles
            ktc = [cpool.tile([128, EB * 512], bf16, tag=f"kt{ch}", name=f"kt{ch}") for ch in range(N_CH)]
            vc = [cpool.tile([128, 4 * VW], bf16, tag=f"va{ch}", name=f"va{ch}") for ch in range(N_CH)]
            qt = cpool.tile([128, EB * 2 * QT], bf16, tag="qt")     # Q^T, RoPE'd
            attn = cpool.tile([64, H * 2 * QT], bf16, tag="attn")   # per-head out

            def rope(dst, src_ps, cos_ap, sin_ap, width):
                """dst(bf16) = rope(src_ps fp32 psum) in [d, s] layout."""
                xb = rpool.tile([128, width], bf16, tag="rope_x")
                nc.scalar.activation(xb[:], src_ps[:], Copy)
                sh = rpool.tile([128, width], bf16, tag="rope_sh")
                nc.vector.stream_shuffle(sh[:], xb[:], PAIRSWAP)
                nc.vector.tensor_mul(xb[:], xb[:], cos_ap)
                nc.vector.tensor_mul(sh[:], sh[:], sin_ap)
                nc.vector.tensor_add(dst, xb[:], sh[:])

            # ---- Q projection + RoPE ----
            wq_sb = load_w("wq")
            for db in range(EB):
                ps = psb.tile([128, 512], f32, tag="ps_proj")
                for eb in range(EB):
                    nc.tensor.matmul(
                        ps[:],
                        wq_sb[:, eb * D_MODEL + db * 128:eb * D_MODEL + db * 128 + 128],
                        xtq[:, eb * 2 * QT:(eb + 1) * 2 * QT],
                        start=(eb == 0),
                        stop=(eb == EB - 1),
                    )
                rope(qt[:, db * 2 * QT:(db + 1) * 2 * QT], ps, cosq[:], sinq[:], 512)

            # ---- K / V projections per 512-column chunk ----
            wk_sb = load_w("wk")
            wv_sb = load_w("wv")
            for ch in range(N_CH):
                xt_t = xtpool.tile([128, EB * 512], bf16, tag="xt_t")
                for eb in range(EB):
                    nc.sync.dma_start(
                        out=xt_t[:, eb * 512:(eb + 1) * 512],
                        in_=xt_d[eb * 128:(eb + 1) * 128, ch * 512:(ch + 1) * 512],
                    )
                ck = cspool.tile([128, 512], bf16, tag="cosk")
                nc.sync.dma_start(out=ck[:], in_=cosk_d[:, ch * 512:(ch + 1) * 512])
                sk = cspool.tile([128, 512], bf16, tag="sink")
                nc.sync.dma_start(out=sk[:], in_=sink_d[:, ch * 512:(ch + 1) * 512])
                # K^T blocks with RoPE
                for db in range(EB):
                    ps = psb.tile([128, 512], f32, tag="ps_proj")
                    for eb in range(EB):
                        nc.tensor.matmul(
                            ps[:],
                            wk_sb[:, eb * D_MODEL + db * 128:eb * D_MODEL + db * 128 + 128],
                            xt_t[:, eb * 512:(eb + 1) * 512],
                            start=(eb == 0),
                            stop=(eb == EB - 1),
                        )
                    rope(
                        ktc[ch][:, db * 512:(db + 1) * 512],
                        ps,
                        ck[:],
                        sk[:],
                        512,
                    )
                # V natural layout, interleaved ones column
                for stl in range(4):
                    st = ch * 4 + stl
                    psa = psb.tile([128, 512], f32, tag="ps_proj")
                    psb2 = pss.tile([128, 256], f32, tag="ps_vb")
                    for eb in range(EB):
                        nc.tensor.matmul(
                            psa[:],
                            xt_t[:, eb * 512 + stl * 128:eb * 512 + stl * 128 + 128],
                            wv_sb[:, eb * D_MODEL:eb * D_MODEL + 512],
                            start=(eb == 0),
                            stop=(eb == EB - 1),
                        )
                    for eb in range(EB):
                        nc.tensor.matmul(
                            psb2[:],
                            xt_t[:, eb * 512 + stl * 128:eb * 512 + stl * 128 + 128],
                            wv_sb[:, eb * D_MODEL + 512:eb * D_MODEL + 768],
                            start=(eb == 0),
                            stop=(eb == EB - 1),
                        )
                    base = stl * VW
                    vtile = vc[ch][:, base:base + VW].rearrange(
                        "p (h d) -> p h d", d=65
                    )
                    nc.vector.memset(vtile[:, :, 64:65], 1.0)
                    nc.vector.tensor_copy(
                        vtile[:, 0:8, 0:64],
                        psa[:].rearrange("p (h d) -> p h d", d=64),
                    )
                    nc.vector.tensor_copy(
                        vtile[:, 8:12, 0:64],
                        psb2[:].rearrange("p (h d) -> p h d", d=64),
                    )

            # ---- attention ----
            for h in range(H):
                kb = h // 2
                ro = 64 * (h % 2)
                for s in range(2):
                    n_pairs = 8 if s == 0 else 16
                    itmap = IT0_MAP if s == 0 else IT1_MAP
                    bias_off = 0 if s == 0 else 8
                    pv = psv.tile([65, QT], f32, tag="ps_pv")
                    for p in range(n_pairs):
                        i0 = itmap[2 * p]
                        i1 = itmap[2 * p + 1]
                        sc = pssc.tile([128, 512], f32, tag="ps_sc")
                        nc.tensor.matmul(
                            sc[:, 0:256],
                            ktc[i0 // 4][ro:ro + 64, kb * 512 + (i0 % 4) * 128:kb * 512 + (i0 % 4) * 128 + 128],
                            qt[ro:ro + 64, kb * 2 * QT + s * QT:kb * 2 * QT + s * QT + QT],
                            start=True,
                            stop=True,
                        )
                        nc.tensor.matmul(
                            sc[:, 256:512],
                            ktc[i1 // 4][ro:ro + 64, kb * 512 + (i1 % 4) * 128:kb * 512 + (i1 % 4) * 128 + 128],
                            qt[ro:ro + 64, kb * 2 * QT + s * QT:kb * 2 * QT + s * QT + QT],
                            start=True,
                            stop=True,
                        )
                        et = epool.tile([128, 512], bf16, tag="et")
                        nc.scalar.activation(
                            et[:],
                            sc[:],
                            Exp,
                            bias=biasp[:, bias_off + p:bias_off + p + 1],
                            scale=0.125,
                        )
                        if p == 0:
                            nc.vector.tensor_mul(et[:], et[:], maskab[:])
                        nc.tensor.matmul(
                            pv[:],
                            vc[i0 // 4][:, (i0 % 4) * VW + h * 65:(i0 % 4) * VW + h * 65 + 65],
                            et[:, 0:256],
                            start=(p == 0),
                            stop=False,
                        )
                        nc.tensor.matmul(
                            pv[:],
                            vc[i1 // 4][:, (i1 % 4) * VW + h * 65:(i1 % 4) * VW + h * 65 + 65],
                            et[:, 256:512],
                            start=False,
                            stop=(p == n_pairs - 1),
                        )
                    # normalize: recip of sums (row 64) -> physical row 0
                    # (partition_broadcast reads the tile's partition 0),
                    # broadcast, multiply
                    rc = npool.tile([128, QT], f32, tag="recip")
                    nc.vector.reciprocal(rc[64:65, :], pv[64:65, :])
                    r0 = npool.tile([1, QT], f32, tag="r0")
                    nc.sync.dma_start(out=r0[:], in_=rc[64:65, :])
                    rb = npool.tile([128, QT], f32, tag="rbcast")
                    nc.gpsimd.partition_broadcast(rb[0:64, :], r0[0:1, :])
                    nc.vector.tensor_mul(
                        attn[0:64, h * 2 * QT + s * QT:h * 2 * QT + s * QT + QT],
                        pv[0:64, :],
                        rb[0:64, :],
                    )

            # ---- output projection (contraction over heads, K=64 each) ----
            # wo resident as two half tiles reusing the freed wq/wk/wv slots
            wo_half = []
            for g in range(2):
                t = wpool.tile([64, 6 * D_MODEL], bf16, tag="w", name=f"wo{g}")
                for j in range(6):
                    h = 6 * g + j
                    nc.sync.dma_start(
                        out=t[:, j * D_MODEL:(j + 1) * D_MODEL],
                        in_=w_d["wo"][h * 64:(h + 1) * 64, :],
                    )
                wo_half.append(t)
            for qtl in range(4):
                po1 = pssc.tile([128, 512], f32, tag="ps_sc")
                po2 = pss.tile([128, 256], f32, tag="ps_vb")
                for h in range(H):
                    lhs = attn[0:64, h * 2 * QT + qtl * 128:h * 2 * QT + qtl * 128 + 128]
                    wo_t = wo_half[h // 6]
                    off = (h % 6) * D_MODEL
                    nc.tensor.matmul(
                        po1[:],
                        lhs,
                        wo_t[:, off:off + 512],
                        start=(h == 0),
                        stop=(h == H - 1),
                    )
                    nc.tensor.matmul(
                        po2[:],
                        lhs,
                        wo_t[:, off + 512:off + 768],
                        start=(h == 0),
                        stop=(h == H - 1),
                    )
                osb = opool.tile([128, D_MODEL], f32, tag="osb")
                nc.vector.tensor_copy(osb[:, 0:512], po1[:])
                nc.vector.tensor_copy(osb[:, 512:768], po2[:])
                nc.sync.dma_start(
                    out=out_d[qtl * 128:(qtl + 1) * 128, :], in_=osb[:]
                )

    nc.compile()
    return nc


_PROGRAM = None


def _get_program():
    global _PROGRAM
    if _PROGRAM is None:
        _PROGRAM = build_program()
    return _PROGRAM


def host_prep(in_features, token_positions, q_proj, k_proj, v_proj, o_proj):
    """Build the 8 per-core input maps."""
    x = np.asarray(in_features, np.float32).reshape(S, D_MODEL)
    tp = np.asarray(token_positions)
    qp = np.asarray(q_proj, np.float32)
    kp = np.asarray(k_proj, np.float32)
    vp = np.asarray(v_proj, np.float32)
    op = np.asarray(o_proj, np.float32)

    xt = np.ascontiguousarray(x.T)                      # [768, 4096] fp32
    xt_bf = xt.astype(BF16)
    wq = np.ascontiguousarray(qp.T).astype(BF16)
    wk = np.ascontiguousarray(kp.T).astype(BF16)
    wv = np.ascontiguousarray(vp.T).astype(BF16)
    wo = np.ascontiguousarray(op.T).astype(BF16)

    inv_freq = 1.0 / THETA ** (np.arange(0, DK, 2, dtype=np.float32) / DK)
    pos = np.clip(tp.astype(np.float32), 0, MAX_SEQ_LEN - 1)
    freq = pos[:, None] * inv_freq[None, :]             # [S, 32]
    cos_t, sin_t = np.cos(freq), np.sin(freq)

    r = np.arange(128)
    fidx = (r % 64) // 2
    sign = np.where(r % 2 == 0, -1.0, 1.0).astype(np.float32)
    cos128 = cos_t[:, fidx].T.astype(np.float32)        # [128, S]
    sin128 = (sin_t[:, fidx].T * sign[:, None]).astype(np.float32)

    # diagonal masks: A = k-tile aligned with q[0:256) first half,
    # B = aligned with second half. scores^T layout: [k(128), q(256)].
    ki = np.arange(128)[:, None]
    qi = np.arange(QT)[None, :]
    mask_a = (ki <= qi).astype(np.float32)
    mask_b = (ki + 128 <= qi).astype(np.float32)
    maskab = np.concatenate([mask_a, mask_b], axis=1).astype(BF16)

    in_maps = []
    for c in range(N_CORES):
        sig = _sigma(c)
        perm = np.concatenate(
            [np.arange(t * 128, (t + 1) * 128) for t in sig]
        )
        qcols = np.concatenate(
            [
                np.arange(QT * c, QT * (c + 1)),
                np.arange(QT * (8 + c), QT * (9 + c)),
            ]
        )
        biasp = np.broadcast_to(_bias_cols(c)[None, :], (128, 24))
        in_maps.append(
            {
                "xt": np.ascontiguousarray(xt_bf[:, perm]),
                "xtq": np.ascontiguousarray(xt_bf[:, qcols]),
                "wq": wq,
                "wk": wk,
                "wv": wv,
                "wo": wo,
                "cosk": np.ascontiguousarray(cos128[:, perm]).astype(BF16),
                "sink": np.ascontiguousarray(sin128[:, perm]).astype(BF16),
                "cosq": np.ascontiguousarray(cos128[:, qcols]).astype(BF16),
                "sinq": np.ascontiguousarray(sin128[:, qcols]).astype(BF16),
                "maskab": maskab,
                "biasp": np.ascontiguousarray(biasp, np.float32),
            }
        )
    return in_maps


def assemble_output(results):
    out = np.empty((1, S, D_MODEL), np.float32)
    for c in range(N_CORES):
        r = np.asarray(results[c]["out"], np.float32)
        out[0, QT * c:QT * (c + 1)] = r[0:QT]
        out[0, QT * (8 + c):QT * (9 + c)] = r[QT:2 * QT]
    return out


def kernel(**inputs):
    from concourse.bass_utils import run_bass_kernel_spmd

    nc = _get_program()
    in_maps = host_prep(**inputs)
    res = run_bass_kernel_spmd(nc, in_maps, list(range(N_CORES)))
    return assemble_output(res.results)


if __name__ == "__main__":
    nc = build_program()
    print("program built and compiled")



# revision 2
# speedup vs baseline: 1.3172x; 1.3172x over previous
"""Trainium2 Bass kernel: causal MultiHeadAttention with RoPE (head-parallel).

B=1, S=4096, D=768, H=12 heads, dk=64, fp32 I/O. 8 NeuronCores, SPMD.

Sharding: head-parallel. Core c owns head A=c and head-slot B=8+c (real for
c<4, zero-weights otherwise). Every core projects K/V/Q for its two heads
over the full sequence (128 partitions = 2x64 dk), runs full-causal
attention for both heads (identical instruction stream on every core, no
cross-core padding), computes the partial output projection
o_part[q, d] = sum_{own heads} attn_h @ Wo_h, and the 8 cores combine
partials with chunked ReduceScatter collectives overlapped with compute.
The host scatters the RS shards into the full output and needs no math.
"""

import sys

if "/opt/trn_rl_repo" not in sys.path:
    sys.path.insert(0, "/opt/trn_rl_repo")

import numpy as np
import ml_dtypes

D_MODEL = 768
H = 12
DK = 64
S = 4096
THETA = 10000.0
MAX_SEQ_LEN = 4096
N_CORES = 8
EB = D_MODEL // 128   # 6 contraction blocks
N_CH = S // 512       # 8 sequence chunks
VW = 130              # V_aug row width per s-tile: 2 heads x (64+ones)

BF16 = ml_dtypes.bfloat16

# ReduceScatter chunks: fire after q-tile T_FIRE, covering q rows [lo, hi).
# Fired one q-tile after the covered rows complete so the collective's
# input-DMA waits are already satisfied and never block the Pool queue.
RS_CHUNKS = [
    (12, 0, 3072),
    (15, 3072, 4096),
]
# out_d row offset of each chunk's shard
RS_OUT_OFF = [0, 384]


def build_program(with_rs=True):
    import concourse.mybir as mybir
    import concourse.tile as tile
    from concourse import bacc, library_config
    from concourse.tile import add_dep_helper

    f32 = mybir.dt.float32
    bf16 = mybir.dt.bfloat16
    Exp = mybir.ActivationFunctionType.Exp
    Copy = mybir.ActivationFunctionType.Copy

    nc = bacc.Bacc(
        "TRN2",
        target_bir_lowering=False,
        debug=False,
        enable_asserts=True,
        num_devices=N_CORES,
    )

    xt_d = nc.dram_tensor("xt", [D_MODEL, S], bf16, kind="ExternalInput")
    w_d = {
        n: nc.dram_tensor(n, [D_MODEL, 128], bf16, kind="ExternalInput")
        for n in ("wq2", "wk2", "wv2")
    }
    wo_d = nc.dram_tensor("wo2", [64, 2 * D_MODEL], bf16, kind="ExternalInput")
    cosk_d = nc.dram_tensor("cosk", [128, S], bf16, kind="ExternalInput")
    sink_d = nc.dram_tensor("sink", [128, S], bf16, kind="ExternalInput")
    mask_d = nc.dram_tensor("maskab", [128, 512], bf16, kind="ExternalInput")
    out_d = nc.dram_tensor("out", [512, D_MODEL], f32, kind="ExternalOutput")
    o_part = nc.dram_tensor("o_part", [S, D_MODEL], bf16, kind="Internal")
    ors = [
        nc.dram_tensor(f"ors{j}", [(hi - lo) // 8, D_MODEL], bf16, kind="Internal")
        for j, (_, lo, hi) in enumerate(RS_CHUNKS)
    ]

    PAIRSWAP = [i ^ 1 for i in range(32)]

    with tile.TileContext(nc) as tc:
        with (
            tc.tile_pool(name="const", bufs=1) as cpool,
            tc.tile_pool(name="rope", bufs=3) as rpool,
            tc.tile_pool(name="expp", bufs=4) as epool,
            tc.tile_pool(name="norm", bufs=4) as npool,
            tc.tile_pool(name="rsrb", bufs=2) as rbpool,
            tc.tile_pool(name="ps", bufs=3, space="PSUM") as pspool,
            tc.tile_pool(name="ps_pv", bufs=2, space="PSUM") as psv,
        ):
            nc.gpsimd.load_library(library_config.attn)

            # ---- persistent tensors; first chunk's inputs loaded first ----
            def load_w(n):
                t = cpool.tile([128, EB * 128], bf16, tag=f"w_{n}", name=n)
                nc.sync.dma_start(
                    out=t[:].rearrange("p (e m) -> p e m", m=128),
                    in_=w_d[n][:].rearrange("(e p) m -> p e m", p=128),
                )
                return t

            def xt_load(lo, hi):
                nc.sync.dma_start(
                    out=xt_sb[:].rearrange("p (e s) -> p e s", s=S)[:, :, lo:hi],
                    in_=xt_d[:].rearrange("(e p) s -> p e s", p=128)[:, :, lo:hi],
                )

            wk_sb = load_w("wk2")
            xt_sb = cpool.tile([128, EB * S], bf16, tag="xt_sb")
            xt_load(0, 512)
            cosk_sb = cpool.tile([128, S], bf16, tag="cosk_sb")
            nc.sync.dma_start(out=cosk_sb[:, 0:512], in_=cosk_d[:, 0:512])
            sink_sb = cpool.tile([128, S], bf16, tag="sink_sb")
            nc.sync.dma_start(out=sink_sb[:, 0:512], in_=sink_d[:, 0:512])
            wq_sb = load_w("wq2")
            maskab = cpool.tile([128, 512], bf16, tag="maskab")
            nc.sync.dma_start(out=maskab[:], in_=mask_d[:])
            wv_sb = load_w("wv2")
            wo_sb = cpool.tile([64, 2 * D_MODEL], bf16, tag="wo2")
            nc.sync.dma_start(out=wo_sb[:], in_=wo_d[:])
            xt_load(512, 1024)
            nc.sync.dma_start(out=cosk_sb[:, 512:S], in_=cosk_d[:, 512:S])
            nc.sync.dma_start(out=sink_sb[:, 512:S], in_=sink_d[:, 512:S])
            xt_load(1024, S)

            ones64 = cpool.tile([65, 64], bf16, tag="ones64")
            nc.vector.memset(ones64[64:65, :], 1.0)
            qt = cpool.tile([128, S], bf16, tag="qt")
            ktc = [
                cpool.tile([128, 512], bf16, tag=f"kt{ch}", name=f"kt{ch}")
                for ch in range(N_CH)
            ]
            vc = [
                cpool.tile([128, 4 * VW], bf16, tag=f"va{ch}", name=f"va{ch}")
                for ch in range(N_CH)
            ]
            attn_sb = [
                cpool.tile([64, S], bf16, tag=f"attn{s}", name=f"attn{s}")
                for s in range(2)
            ]

            def rope(dst, src_ps, cos_ap, sin_ap):
                xb = rpool.tile([128, 512], bf16, tag="rope_x")
                nc.vector.tensor_copy(xb[:], src_ps[:])
                sh = rpool.tile([128, 512], bf16, tag="rope_sh")
                nc.vector.stream_shuffle(sh[:], xb[:], PAIRSWAP)
                nc.vector.tensor_mul(xb[:], xb[:], cos_ap)
                nc.vector.tensor_mul(sh[:], sh[:], sin_ap)
                nc.vector.tensor_add(dst, xb[:], sh[:])

            def attention_tile(slot, T):
                """Score/exp/PV matmuls for one (head-slot, 256-row q-tile).

                Software-pipelined: group g+1's score matmuls are issued
                before group g's PV matmuls so the PE never sits behind the
                exp. Normalization is issued later (see finalize_tile)."""
                ro = 64 * slot
                qslice = qt[ro:ro + 64, T * 256:T * 256 + 256]
                pv = psv.tile([65, 256], f32, tag="ps_pv")
                groups = [(pg, 2 if pg + 1 <= T else 1) for pg in range(0, T + 1, 2)]

                def issue_pv(pg, w, et):
                    for pi in range(w):
                        p = pg + pi
                        for j in range(2):
                            t = 2 * p + j
                            nc.tensor.matmul(
                                pv[:],
                                vc[t // 4][:, (t % 4) * VW + slot * 65:(t % 4) * VW + slot * 65 + 65],
                                et[:, (2 * pi + j) * 256:(2 * pi + j + 1) * 256],
                                start=(p == 0 and j == 0),
                                stop=(p == T and j == 1),
                            )

                prev = None
                for pg, w in groups:
                    sc = pspool.tile([128, 1024], f32, tag="ps")
                    for pi in range(w):
                        for j in range(2):
                            t = 2 * (pg + pi) + j
                            nc.tensor.matmul(
                                sc[:, (2 * pi + j) * 256:(2 * pi + j + 1) * 256],
                                ktc[t // 4][ro:ro + 64, (t % 4) * 128:(t % 4) * 128 + 128],
                                qslice,
                                start=True,
                                stop=True,
                            )
                    et = epool.tile([128, 1024], bf16, tag="et")
                    nc.scalar.activation(
                        et[:, 0:512 * w], sc[:, 0:512 * w], Exp, bias=0.0, scale=0.125
                    )
                    if pg + w - 1 == T:  # group holds the diagonal pair
                        off = 512 * (w - 1)
                        nc.vector.tensor_mul(
                            et[:, off:off + 512], et[:, off:off + 512], maskab[:]
                        )
                    if prev is not None:
                        issue_pv(*prev)
                    prev = (pg, w, et)
                issue_pv(*prev)
                # reduce + reciprocal now; broadcast and normalize deferred
                pvs = npool.tile([65, 256], f32, tag="pvs")
                nc.vector.tensor_copy(pvs[:], pv[:])
                rrow = npool.tile([65, 256], bf16, tag="rrow")
                with nc.allow_low_precision(reason="bf16 softmax denominators"):
                    nc.vector.reciprocal(rrow[64:65, :], pvs[64:65, :])
                return pvs, rrow

            def finalize_tile(T, handles):
                """Deferred normalize (broadcast via K=1 matmul) + output
                projection for q-tile T; issued one tile later so the
                reciprocal is ready and the PE never waits."""
                for slot in range(2):
                    pvs, rrow = handles[slot]
                    rb = psv.tile([65, 256], f32, tag="ps_pv")
                    nc.tensor.matmul(
                        rb[0:64, :], ones64[64:65, :], rrow[64:65, :],
                        start=True, stop=True,
                    )
                    nc.vector.tensor_mul(
                        attn_sb[slot][0:64, T * 256:T * 256 + 256],
                        pvs[0:64, :],
                        rb[0:64, :],
                    )
                o_proj_pair(T)

            def o_proj_pair(T):
                osb = rbpool.tile([128, 2 * D_MODEL], bf16, tag="osb")
                for half in range(2):
                    qtl = 2 * T + half
                    pot = pspool.tile([128, 1024], f32, tag="ps")
                    po = pot[:, 0:512]
                    po2 = pot[:, 512:768]
                    for slot in range(2):
                        lhsT = attn_sb[slot][0:64, qtl * 128:qtl * 128 + 128]
                        nc.tensor.matmul(
                            po,
                            lhsT,
                            wo_sb[:, slot * D_MODEL:slot * D_MODEL + 512],
                            start=(slot == 0),
                            stop=(slot == 1),
                        )
                        nc.tensor.matmul(
                            po2,
                            lhsT,
                            wo_sb[:, slot * D_MODEL + 512:slot * D_MODEL + 768],
                            start=(slot == 0),
                            stop=(slot == 1),
                        )
                    nc.vector.tensor_copy(
                        osb[:, half * D_MODEL:(half + 1) * D_MODEL],
                        pot[:, 0:768],
                    )
                last_opart[0] = nc.sync.dma_start(
                    out=o_part[T * 256:(T + 1) * 256, :].rearrange(
                        "(h p) d -> p h d", p=128
                    ),
                    in_=osb[:].rearrange("p (h d) -> p h d", d=D_MODEL),
                )

            def fire_rs(j):
                _, lo, hi = RS_CHUNKS[j]
                nc.gpsimd.collective_compute(
                    "ReduceScatter",
                    mybir.AluOpType.add,
                    replica_groups=[list(range(N_CORES))],
                    ins=[o_part[lo:hi, :]],
                    outs=[ors[j][:]],
                )

            def readback(j):
                _, lo, hi = RS_CHUNKS[j]
                shard = (hi - lo) // 8
                for b in range(0, shard, 128):
                    bb = min(128, shard - b)
                    rt = rbpool.tile([128, D_MODEL], bf16, tag="rt")
                    rd = nc.sync.dma_start(out=rt[0:bb, :], in_=ors[j][b:b + bb, :])
                    # keep readback DMAs after all o_part writes so the
                    # round-robin DMA-queue counts of collective waits never
                    # include collective-dependent transfers
                    if last_opart[0] is not None:
                        add_dep_helper(
                            rd.ins, last_opart[0].ins, sync=True,
                            reason="readback after o_part stream",
                        )
                    rtf = rbpool.tile([128, D_MODEL], f32, tag="rtf")
                    nc.scalar.activation(rtf[0:bb, :], rt[0:bb, :], Copy)
                    nc.sync.dma_start(
                        out=out_d[RS_OUT_OFF[j] + b:RS_OUT_OFF[j] + b + bb, :],
                        in_=rtf[0:bb, :],
                    )

            # ---- main loop: projection chunk ch, then attention q-tiles;
            # tile finalization (normalize + o_proj + RS) runs one tile behind
            rs_next = 0
            last_opart = [None]
            pending = None  # (T, handles)

            def do_proj_kq(ch):
                def xt_t_slice(eb, lo, hi):
                    return xt_sb[:, eb * S + ch * 512 + lo:eb * S + ch * 512 + hi]
                ck = cosk_sb[:, ch * 512:(ch + 1) * 512]
                sk = sink_sb[:, ch * 512:(ch + 1) * 512]

                # K^T and Q^T share one PSUM tile; RoPE applied to both
                psKQ = pspool.tile([128, 1024], f32, tag="ps")
                for eb in range(EB):
                    nc.tensor.matmul(
                        psKQ[:, 0:512],
                        wk_sb[:, eb * 128:(eb + 1) * 128],
                        xt_t_slice(eb, 0, 512),
                        start=(eb == 0),
                        stop=(eb == EB - 1),
                    )
                for eb in range(EB):
                    nc.tensor.matmul(
                        psKQ[:, 512:1024],
                        wq_sb[:, eb * 128:(eb + 1) * 128],
                        xt_t_slice(eb, 0, 512),
                        start=(eb == 0),
                        stop=(eb == EB - 1),
                    )
                rope(ktc[ch][:], psKQ[:, 0:512], ck, sk)
                rope(qt[:, ch * 512:(ch + 1) * 512], psKQ[:, 512:1024], ck, sk)

            def do_proj_v(ch):
                def xt_t_slice(eb, lo, hi):
                    return xt_sb[:, eb * S + ch * 512 + lo:eb * S + ch * 512 + hi]
                # V (natural layout, interleaved ones column per head):
                # 4 s-tiles accumulate into one PSUM tile
                psV4 = pspool.tile([128, 1024], f32, tag="ps")
                for stl in range(4):
                    for eb in range(EB):
                        nc.tensor.matmul(
                            psV4[:, stl * 256:stl * 256 + 128],
                            xt_t_slice(eb, stl * 128, stl * 128 + 128),
                            wv_sb[:, eb * 128:(eb + 1) * 128],
                            start=(eb == 0),
                            stop=(eb == EB - 1),
                        )
                for stl in range(4):
                    vtile = vc[ch][:, stl * VW:(stl + 1) * VW].rearrange(
                        "p (h d) -> p h d", d=65
                    )
                    nc.vector.memset(vtile[:, :, 64:65], 1.0)
                    nc.vector.tensor_copy(
                        vtile[:, :, 0:64],
                        psV4[:, stl * 256:stl * 256 + 128].rearrange(
                            "p (h d) -> p h d", d=64
                        ),
                    )

            # projections run one chunk ahead, split and issued mid-chunk so
            # the attention tiles keep the Act engine fed at boundaries
            do_proj_kq(0)
            do_proj_v(0)
            for ch in range(N_CH):
                for T in (2 * ch, 2 * ch + 1):
                    handles = [attention_tile(slot, T) for slot in range(2)]
                    if pending is not None:
                        pT, ph = pending
                        finalize_tile(pT, ph)
                        if (
                            with_rs
                            and rs_next < len(RS_CHUNKS)
                            and RS_CHUNKS[rs_next][0] == pT
                        ):
                            fire_rs(rs_next)
                            rs_next += 1
                    pending = (T, handles)
                    if ch + 1 < N_CH:
                        if T == 2 * ch:
                            do_proj_kq(ch + 1)
                        else:
                            do_proj_v(ch + 1)
            finalize_tile(*pending)
            while with_rs and rs_next < len(RS_CHUNKS):
                fire_rs(rs_next)
                rs_next += 1
            if with_rs:
                for j in range(len(RS_CHUNKS)):
                    readback(j)

    nc.compile()
    return nc


_PROGRAM = None


def _get_program():
    global _PROGRAM
    if _PROGRAM is None:
        _PROGRAM = build_program()
    return _PROGRAM


def host_prep(in_features, token_positions, q_proj, k_proj, v_proj, o_proj):
    """Build the 8 per-core input maps."""
    x = np.asarray(in_features, np.float32).reshape(S, D_MODEL)
    tp = np.asarray(token_positions)
    qp = np.asarray(q_proj, np.float32)
    kp = np.asarray(k_proj, np.float32)
    vp = np.asarray(v_proj, np.float32)
    op = np.asarray(o_proj, np.float32)

    xt_bf = np.ascontiguousarray(x.T).astype(BF16)      # [768, 4096]
    wqT = np.ascontiguousarray(qp.T)                    # [in 768, out 768]
    wkT = np.ascontiguousarray(kp.T)
    wvT = np.ascontiguousarray(vp.T)
    opT = np.ascontiguousarray(op.T)                    # [in-dk 768, out 768]

    inv_freq = 1.0 / THETA ** (np.arange(0, DK, 2, dtype=np.float32) / DK)
    pos = np.clip(tp.astype(np.float32), 0, MAX_SEQ_LEN - 1)
    freq = pos[:, None] * inv_freq[None, :]             # [S, 32]
    cos_t, sin_t = np.cos(freq), np.sin(freq)

    r = np.arange(128)
    fidx = (r % 64) // 2
    sign = np.where(r % 2 == 0, -1.0, 1.0).astype(np.float32)
    cos128 = cos_t[:, fidx].T.astype(BF16)              # [128, S]
    sin128 = (sin_t[:, fidx].T * sign[:, None]).astype(BF16)

    ki = np.arange(128)[:, None]
    qi = np.arange(256)[None, :]
    mask_a = (ki <= qi).astype(np.float32)
    mask_b = (ki + 128 <= qi).astype(np.float32)
    maskab = np.concatenate([mask_a, mask_b], axis=1).astype(BF16)

    in_maps = []
    for c in range(N_CORES):
        hA = c
        hB = 8 + c if c < 4 else None

        def wslice(wT):
            out = np.zeros((D_MODEL, 128), np.float32)
            out[:, 0:64] = wT[:, hA * 64:(hA + 1) * 64]
            if hB is not None:
                out[:, 64:128] = wT[:, hB * 64:(hB + 1) * 64]
            return out.astype(BF16)

        wo2 = np.zeros((64, 2 * D_MODEL), np.float32)
        wo2[:, 0:D_MODEL] = opT[hA * 64:(hA + 1) * 64, :]
        if hB is not None:
            wo2[:, D_MODEL:] = opT[hB * 64:(hB + 1) * 64, :]

        in_maps.append(
            {
                "xt": xt_bf,
                "wq2": wslice(wqT),
                "wk2": wslice(wkT),
                "wv2": wslice(wvT),
                "wo2": wo2.astype(BF16),
                "cosk": cos128,
                "sink": sin128,
                "maskab": maskab,
            }
        )
    return in_maps


def assemble_output(results):
    out = np.empty((1, S, D_MODEL), np.float32)
    for c in range(N_CORES):
        r = np.asarray(results[c]["out"], np.float32)
        for j, (_, lo, hi) in enumerate(RS_CHUNKS):
            shard = (hi - lo) // 8
            oo = RS_OUT_OFF[j]
            out[0, lo + shard * c:lo + shard * (c + 1)] = r[oo:oo + shard]
    return out


def kernel(**inputs):
    from concourse.bass_utils import run_bass_kernel_spmd

    nc = _get_program()
    in_maps = host_prep(**inputs)
    res = run_bass_kernel_spmd(nc, in_maps, list(range(N_CORES)))
    return assemble_output(res.results)


if __name__ == "__main__":
    nc = build_program()
    print("program built and compiled")


# revision 3
# speedup vs baseline: 1.3179x; 1.0005x over previous
"""Trainium2 Bass kernel: causal MultiHeadAttention with RoPE (head-parallel).

B=1, S=4096, D=768, H=12 heads, dk=64, fp32 I/O. 8 NeuronCores, SPMD.

Sharding: head-parallel. Core c owns head A=c and head-slot B=8+c (real for
c<4, zero-weights otherwise). Every core projects K/V/Q for its two heads
over the full sequence (128 partitions = 2x64 dk), runs full-causal
attention for both heads (identical instruction stream on every core, no
cross-core padding), computes the partial output projection
o_part[q, d] = sum_{own heads} attn_h @ Wo_h, and the 8 cores combine
partials with chunked ReduceScatter collectives overlapped with compute.
The host scatters the RS shards into the full output and needs no math.
"""

import sys

if "/opt/trn_rl_repo" not in sys.path:
    sys.path.insert(0, "/opt/trn_rl_repo")

import numpy as np
import ml_dtypes

D_MODEL = 768
H = 12
DK = 64
S = 4096
THETA = 10000.0
MAX_SEQ_LEN = 4096
N_CORES = 8
EB = D_MODEL // 128   # 6 contraction blocks
N_CH = S // 512       # 8 sequence chunks
VW = 130              # V_aug row width per s-tile: 2 heads x (64+ones)

BF16 = ml_dtypes.bfloat16

# ReduceScatter chunks: fire after q-tile T_FIRE, covering q rows [lo, hi).
# Fired one q-tile after the covered rows complete so the collective's
# input-DMA waits are already satisfied and never block the Pool queue.
RS_CHUNKS = [
    (12, 0, 3072),
    (15, 3072, 4096),
]
# out_d row offset of each chunk's shard
RS_OUT_OFF = [0, 384]


def build_program(with_rs=True):
    import concourse.mybir as mybir
    import concourse.tile as tile
    from concourse import bacc, library_config
    from concourse.tile import add_dep_helper

    f32 = mybir.dt.float32
    bf16 = mybir.dt.bfloat16
    Exp = mybir.ActivationFunctionType.Exp
    Copy = mybir.ActivationFunctionType.Copy

    nc = bacc.Bacc(
        "TRN2",
        target_bir_lowering=False,
        debug=False,
        enable_asserts=True,
        num_devices=N_CORES,
    )

    xt_d = nc.dram_tensor("xt", [D_MODEL, S], bf16, kind="ExternalInput")
    w_d = {
        n: nc.dram_tensor(n, [D_MODEL, 128], bf16, kind="ExternalInput")
        for n in ("wq2", "wk2", "wv2")
    }
    wo_d = nc.dram_tensor("wo2", [64, 2 * D_MODEL], bf16, kind="ExternalInput")
    cosk_d = nc.dram_tensor("cosk", [128, S], bf16, kind="ExternalInput")
    sink_d = nc.dram_tensor("sink", [128, S], bf16, kind="ExternalInput")
    mask_d = nc.dram_tensor("maskab", [128, 512], bf16, kind="ExternalInput")
    out_d = nc.dram_tensor("out", [512, D_MODEL], f32, kind="ExternalOutput")
    o_part = nc.dram_tensor("o_part", [S, D_MODEL], bf16, kind="Internal")
    ors = [
        nc.dram_tensor(f"ors{j}", [(hi - lo) // 8, D_MODEL], bf16, kind="Internal")
        for j, (_, lo, hi) in enumerate(RS_CHUNKS)
    ]

    PAIRSWAP = [i ^ 1 for i in range(32)]

    with tile.TileContext(nc) as tc:
        with (
            tc.tile_pool(name="const", bufs=1) as cpool,
            tc.tile_pool(name="rope", bufs=4) as rpool,
            tc.tile_pool(name="expp", bufs=6) as epool,
            tc.tile_pool(name="norm", bufs=6) as npool,
            tc.tile_pool(name="rsrb", bufs=3) as rbpool,
            tc.tile_pool(name="ps", bufs=3, space="PSUM") as pspool,
            tc.tile_pool(name="ps_pv", bufs=2, space="PSUM") as psv,
        ):
            nc.gpsimd.load_library(library_config.attn)

            # ---- persistent tensors; first chunk's inputs loaded first ----
            def load_w(n):
                t = cpool.tile([128, EB * 128], bf16, tag=f"w_{n}", name=n)
                nc.sync.dma_start(
                    out=t[:].rearrange("p (e m) -> p e m", m=128),
                    in_=w_d[n][:].rearrange("(e p) m -> p e m", p=128),
                )
                return t

            def xt_load(lo, hi):
                nc.sync.dma_start(
                    out=xt_sb[:].rearrange("p (e s) -> p e s", s=S)[:, :, lo:hi],
                    in_=xt_d[:].rearrange("(e p) s -> p e s", p=128)[:, :, lo:hi],
                )

            wk_sb = load_w("wk2")
            xt_sb = cpool.tile([128, EB * S], bf16, tag="xt_sb")
            xt_load(0, 512)
            cosk_sb = cpool.tile([128, S], bf16, tag="cosk_sb")
            nc.sync.dma_start(out=cosk_sb[:, 0:512], in_=cosk_d[:, 0:512])
            sink_sb = cpool.tile([128, S], bf16, tag="sink_sb")
            nc.sync.dma_start(out=sink_sb[:, 0:512], in_=sink_d[:, 0:512])
            wq_sb = load_w("wq2")
            maskab = cpool.tile([128, 512], bf16, tag="maskab")
            nc.sync.dma_start(out=maskab[:], in_=mask_d[:])
            wv_sb = load_w("wv2")
            wo_sb = cpool.tile([64, 2 * D_MODEL], bf16, tag="wo2")
            nc.sync.dma_start(out=wo_sb[:], in_=wo_d[:])
            xt_load(512, 1024)
            nc.sync.dma_start(out=cosk_sb[:, 512:S], in_=cosk_d[:, 512:S])
            nc.sync.dma_start(out=sink_sb[:, 512:S], in_=sink_d[:, 512:S])
            xt_load(1024, S)

            ones64 = cpool.tile([65, 64], bf16, tag="ones64")
            nc.vector.memset(ones64[64:65, :], 1.0)
            qt = cpool.tile([128, S], bf16, tag="qt")
            ktc = [
                cpool.tile([128, 512], bf16, tag=f"kt{ch}", name=f"kt{ch}")
                for ch in range(N_CH)
            ]
            vc = [
                cpool.tile([128, 4 * VW], bf16, tag=f"va{ch}", name=f"va{ch}")
                for ch in range(N_CH)
            ]
            attn_sb = [
                cpool.tile([64, S], bf16, tag=f"attn{s}", name=f"attn{s}")
                for s in range(2)
            ]

            def rope(dst, src_ps, cos_ap, sin_ap):
                xb = rpool.tile([128, 512], bf16, tag="rope_x")
                nc.vector.tensor_copy(xb[:], src_ps[:])
                sh = rpool.tile([128, 512], bf16, tag="rope_sh")
                nc.vector.stream_shuffle(sh[:], xb[:], PAIRSWAP)
                nc.vector.tensor_mul(xb[:], xb[:], cos_ap)
                nc.vector.tensor_mul(sh[:], sh[:], sin_ap)
                nc.vector.tensor_add(dst, xb[:], sh[:])

            def attention_tile(slot, T):
                """Score/exp/PV matmuls for one (head-slot, 256-row q-tile).

                Software-pipelined: group g+1's score matmuls are issued
                before group g's PV matmuls so the PE never sits behind the
                exp. Normalization is issued later (see finalize_tile)."""
                ro = 64 * slot
                qslice = qt[ro:ro + 64, T * 256:T * 256 + 256]
                pv = psv.tile([65, 256], f32, tag="ps_pv")
                groups = [(pg, 2 if pg + 1 <= T else 1) for pg in range(0, T + 1, 2)]

                def issue_pv(pg, w, et):
                    for pi in range(w):
                        p = pg + pi
                        for j in range(2):
                            t = 2 * p + j
                            nc.tensor.matmul(
                                pv[:],
                                vc[t // 4][:, (t % 4) * VW + slot * 65:(t % 4) * VW + slot * 65 + 65],
                                et[:, (2 * pi + j) * 256:(2 * pi + j + 1) * 256],
                                start=(p == 0 and j == 0),
                                stop=(p == T and j == 1),
                            )

                prev = None
                for pg, w in groups:
                    sc = pspool.tile([128, 1024], f32, tag="ps")
                    for pi in range(w):
                        for j in range(2):
                            t = 2 * (pg + pi) + j
                            nc.tensor.matmul(
                                sc[:, (2 * pi + j) * 256:(2 * pi + j + 1) * 256],
                                ktc[t // 4][ro:ro + 64, (t % 4) * 128:(t % 4) * 128 + 128],
                                qslice,
                                start=True,
                                stop=True,
                            )
                    et = epool.tile([128, 1024], bf16, tag="et")
                    nc.scalar.activation(
                        et[:, 0:512 * w], sc[:, 0:512 * w], Exp, bias=0.0, scale=0.125
                    )
                    if pg + w - 1 == T:  # group holds the diagonal pair
                        off = 512 * (w - 1)
                        nc.vector.tensor_mul(
                            et[:, off:off + 512], et[:, off:off + 512], maskab[:]
                        )
                    if prev is not None:
                        issue_pv(*prev)
                    prev = (pg, w, et)
                issue_pv(*prev)
                # reduce + reciprocal now; broadcast and normalize deferred
                pvs = npool.tile([65, 256], f32, tag="pvs")
                nc.vector.tensor_copy(pvs[:], pv[:])
                rrow = npool.tile([65, 256], bf16, tag="rrow")
                with nc.allow_low_precision(reason="bf16 softmax denominators"):
                    nc.vector.reciprocal(rrow[64:65, :], pvs[64:65, :])
                return pvs, rrow

            def finalize_tile(T, handles):
                """Deferred normalize (broadcast via K=1 matmul) + output
                projection for q-tile T; issued one tile later so the
                reciprocal is ready and the PE never waits."""
                for slot in range(2):
                    pvs, rrow = handles[slot]
                    rb = psv.tile([65, 256], f32, tag="ps_pv")
                    nc.tensor.matmul(
                        rb[0:64, :], ones64[64:65, :], rrow[64:65, :],
                        start=True, stop=True,
                    )
                    nc.vector.tensor_mul(
                        attn_sb[slot][0:64, T * 256:T * 256 + 256],
                        pvs[0:64, :],
                        rb[0:64, :],
                    )
                o_proj_pair(T)

            def o_proj_pair(T):
                osb = rbpool.tile([128, 2 * D_MODEL], bf16, tag="osb")
                for half in range(2):
                    qtl = 2 * T + half
                    pot = pspool.tile([128, 1024], f32, tag="ps")
                    po = pot[:, 0:512]
                    po2 = pot[:, 512:768]
                    for slot in range(2):
                        lhsT = attn_sb[slot][0:64, qtl * 128:qtl * 128 + 128]
                        nc.tensor.matmul(
                            po,
                            lhsT,
                            wo_sb[:, slot * D_MODEL:slot * D_MODEL + 512],
                            start=(slot == 0),
                            stop=(slot == 1),
                        )
                        nc.tensor.matmul(
                            po2,
                            lhsT,
                            wo_sb[:, slot * D_MODEL + 512:slot * D_MODEL + 768],
                            start=(slot == 0),
                            stop=(slot == 1),
                        )
                    nc.vector.tensor_copy(
                        osb[:, half * D_MODEL:(half + 1) * D_MODEL],
                        pot[:, 0:768],
                    )
                last_opart[0] = nc.sync.dma_start(
                    out=o_part[T * 256:(T + 1) * 256, :].rearrange(
                        "(h p) d -> p h d", p=128
                    ),
                    in_=osb[:].rearrange("p (h d) -> p h d", d=D_MODEL),
                )

            def fire_rs(j):
                _, lo, hi = RS_CHUNKS[j]
                nc.gpsimd.collective_compute(
                    "ReduceScatter",
                    mybir.AluOpType.add,
                    replica_groups=[list(range(N_CORES))],
                    ins=[o_part[lo:hi, :]],
                    outs=[ors[j][:]],
                )

            def readback(j):
                _, lo, hi = RS_CHUNKS[j]
                shard = (hi - lo) // 8
                for b in range(0, shard, 128):
                    bb = min(128, shard - b)
                    rt = rbpool.tile([128, D_MODEL], bf16, tag="rt")
                    rd = nc.sync.dma_start(out=rt[0:bb, :], in_=ors[j][b:b + bb, :])
                    # keep readback DMAs after all o_part writes so the
                    # round-robin DMA-queue counts of collective waits never
                    # include collective-dependent transfers
                    if last_opart[0] is not None:
                        add_dep_helper(
                            rd.ins, last_opart[0].ins, sync=True,
                            reason="readback after o_part stream",
                        )
                    rtf = rbpool.tile([128, D_MODEL], f32, tag="rtf")
                    nc.scalar.activation(rtf[0:bb, :], rt[0:bb, :], Copy)
                    nc.sync.dma_start(
                        out=out_d[RS_OUT_OFF[j] + b:RS_OUT_OFF[j] + b + bb, :],
                        in_=rtf[0:bb, :],
                    )

            # ---- main loop: projection chunk ch, then attention q-tiles;
            # tile finalization (normalize + o_proj + RS) runs one tile behind
            rs_next = 0
            last_opart = [None]
            pending = None  # (T, handles)

            def do_proj_kq(ch):
                def xt_t_slice(eb, lo, hi):
                    return xt_sb[:, eb * S + ch * 512 + lo:eb * S + ch * 512 + hi]
                ck = cosk_sb[:, ch * 512:(ch + 1) * 512]
                sk = sink_sb[:, ch * 512:(ch + 1) * 512]

                # K^T and Q^T share one PSUM tile; RoPE applied to both
                psKQ = pspool.tile([128, 1024], f32, tag="ps")
                for eb in range(EB):
                    nc.tensor.matmul(
                        psKQ[:, 0:512],
                        wk_sb[:, eb * 128:(eb + 1) * 128],
                        xt_t_slice(eb, 0, 512),
                        start=(eb == 0),
                        stop=(eb == EB - 1),
                    )
                for eb in range(EB):
                    nc.tensor.matmul(
                        psKQ[:, 512:1024],
                        wq_sb[:, eb * 128:(eb + 1) * 128],
                        xt_t_slice(eb, 0, 512),
                        start=(eb == 0),
                        stop=(eb == EB - 1),
                    )
                rope(ktc[ch][:], psKQ[:, 0:512], ck, sk)
                rope(qt[:, ch * 512:(ch + 1) * 512], psKQ[:, 512:1024], ck, sk)

            def do_proj_v(ch):
                def xt_t_slice(eb, lo, hi):
                    return xt_sb[:, eb * S + ch * 512 + lo:eb * S + ch * 512 + hi]
                # V (natural layout, interleaved ones column per head):
                # 4 s-tiles accumulate into one PSUM tile
                psV4 = pspool.tile([128, 1024], f32, tag="ps")
                for stl in range(4):
                    for eb in range(EB):
                        nc.tensor.matmul(
                            psV4[:, stl * 256:stl * 256 + 128],
                            xt_t_slice(eb, stl * 128, stl * 128 + 128),
                            wv_sb[:, eb * 128:(eb + 1) * 128],
                            start=(eb == 0),
                            stop=(eb == EB - 1),
                        )
                for stl in range(4):
                    vtile = vc[ch][:, stl * VW:(stl + 1) * VW].rearrange(
                        "p (h d) -> p h d", d=65
                    )
                    nc.vector.memset(vtile[:, :, 64:65], 1.0)
                    nc.vector.tensor_copy(
                        vtile[:, :, 0:64],
                        psV4[:, stl * 256:stl * 256 + 128].rearrange(
                            "p (h d) -> p h d", d=64
                        ),
                    )

            # projections run one chunk ahead, split and issued mid-chunk so
            # the attention tiles keep the Act engine fed at boundaries
            do_proj_kq(0)
            do_proj_v(0)
            for ch in range(N_CH):
                for T in (2 * ch, 2 * ch + 1):
                    handles = [attention_tile(slot, T) for slot in range(2)]
                    if pending is not None:
                        pT, ph = pending
                        finalize_tile(pT, ph)
                        if (
                            with_rs
                            and rs_next < len(RS_CHUNKS)
                            and RS_CHUNKS[rs_next][0] == pT
                        ):
                            fire_rs(rs_next)
                            rs_next += 1
                    pending = (T, handles)
                    if ch + 1 < N_CH:
                        if T == 2 * ch:
                            do_proj_kq(ch + 1)
                        else:
                            do_proj_v(ch + 1)
            finalize_tile(*pending)
            while with_rs and rs_next < len(RS_CHUNKS):
                fire_rs(rs_next)
                rs_next += 1
            if with_rs:
                for j in range(len(RS_CHUNKS)):
                    readback(j)

    nc.compile()
    return nc


_PROGRAM = None


def _get_program():
    global _PROGRAM
    if _PROGRAM is None:
        _PROGRAM = build_program()
    return _PROGRAM


def host_prep(in_features, token_positions, q_proj, k_proj, v_proj, o_proj):
    """Build the 8 per-core input maps."""
    x = np.asarray(in_features, np.float32).reshape(S, D_MODEL)
    tp = np.asarray(token_positions)
    qp = np.asarray(q_proj, np.float32)
    kp = np.asarray(k_proj, np.float32)
    vp = np.asarray(v_proj, np.float32)
    op = np.asarray(o_proj, np.float32)

    xt_bf = np.ascontiguousarray(x.T).astype(BF16)      # [768, 4096]
    wqT = np.ascontiguousarray(qp.T)                    # [in 768, out 768]
    wkT = np.ascontiguousarray(kp.T)
    wvT = np.ascontiguousarray(vp.T)
    opT = np.ascontiguousarray(op.T)                    # [in-dk 768, out 768]

    inv_freq = 1.0 / THETA ** (np.arange(0, DK, 2, dtype=np.float32) / DK)
    pos = np.clip(tp.astype(np.float32), 0, MAX_SEQ_LEN - 1)
    freq = pos[:, None] * inv_freq[None, :]             # [S, 32]
    cos_t, sin_t = np.cos(freq), np.sin(freq)

    r = np.arange(128)
    fidx = (r % 64) // 2
    sign = np.where(r % 2 == 0, -1.0, 1.0).astype(np.float32)
    cos128 = cos_t[:, fidx].T.astype(BF16)              # [128, S]
    sin128 = (sin_t[:, fidx].T * sign[:, None]).astype(BF16)

    ki = np.arange(128)[:, None]
    qi = np.arange(256)[None, :]
    mask_a = (ki <= qi).astype(np.float32)
    mask_b = (ki + 128 <= qi).astype(np.float32)
    maskab = np.concatenate([mask_a, mask_b], axis=1).astype(BF16)

    in_maps = []
    for c in range(N_CORES):
        hA = c
        hB = 8 + c if c < 4 else None

        def wslice(wT):
            out = np.zeros((D_MODEL, 128), np.float32)
            out[:, 0:64] = wT[:, hA * 64:(hA + 1) * 64]
            if hB is not None:
                out[:, 64:128] = wT[:, hB * 64:(hB + 1) * 64]
            return out.astype(BF16)

        wo2 = np.zeros((64, 2 * D_MODEL), np.float32)
        wo2[:, 0:D_MODEL] = opT[hA * 64:(hA + 1) * 64, :]
        if hB is not None:
            wo2[:, D_MODEL:] = opT[hB * 64:(hB + 1) * 64, :]

        in_maps.append(
            {
                "xt": xt_bf,
                "wq2": wslice(wqT),
                "wk2": wslice(wkT),
                "wv2": wslice(wvT),
                "wo2": wo2.astype(BF16),
                "cosk": cos128,
                "sink": sin128,
                "maskab": maskab,
            }
        )
    return in_maps


def assemble_output(results):
    out = np.empty((1, S, D_MODEL), np.float32)
    for c in range(N_CORES):
        r = np.asarray(results[c]["out"], np.float32)
        for j, (_, lo, hi) in enumerate(RS_CHUNKS):
            shard = (hi - lo) // 8
            oo = RS_OUT_OFF[j]
            out[0, lo + shard * c:lo + shard * (c + 1)] = r[oo:oo + shard]
    return out


def kernel(**inputs):
    from concourse.bass_utils import run_bass_kernel_spmd

    nc = _get_program()
    in_maps = host_prep(**inputs)
    res = run_bass_kernel_spmd(nc, in_maps, list(range(N_CORES)))
    return assemble_output(res.results)


if __name__ == "__main__":
    nc = build_program()
    print("program built and compiled")


# revision 4
# speedup vs baseline: 1.3641x; 1.0351x over previous
"""Trainium2 Bass kernel: causal MultiHeadAttention with RoPE (head-parallel).

B=1, S=4096, D=768, H=12 heads, dk=64, fp32 I/O. 8 NeuronCores, SPMD.

Sharding: head-parallel with split tail heads. Core c owns whole head
A=c (heads 0..7) plus HALF of head B=8+c//2: the q-tiles of parity c%2
(4 tiles of 256 rows each). Every core projects K/V/Q for its two heads
over the full sequence, runs full-causal attention for head A and its
four B q-tiles (identical instruction stream everywhere; the lone
parity-dependent bit is a [128,1024] mask input), computes bf16 partial
output projections, and combines them with ReduceScatter collectives:
an 8-way RS for the A-partials (q-sharded result) and a 4-way RS over
each parity group for the B-partials, which the host adds on top.
"""

import sys

if "/opt/trn_rl_repo" not in sys.path:
    sys.path.insert(0, "/opt/trn_rl_repo")

import numpy as np
import ml_dtypes

D_MODEL = 768
H = 12
DK = 64
S = 4096
THETA = 10000.0
MAX_SEQ_LEN = 4096
N_CORES = 8
EB = D_MODEL // 128   # 6 contraction blocks
N_CH = S // 512       # 8 sequence chunks
VW = 130              # V_aug row width per s-tile: 2 heads x (64+ones)

BF16 = ml_dtypes.bfloat16

# ReduceScatter chunks: fire after q-tile T_FIRE, covering q rows [lo, hi).
# Fired one q-tile after the covered rows complete so the collective's
# input-DMA waits are already satisfied and never block the Pool queue.
RS_CHUNKS = [
    (12, 0, 3072),
    (15, 3072, 4096),
]
# out_d row offset of each chunk's shard
RS_OUT_OFF = [0, 384]


def build_program(with_rs=True):
    import concourse.mybir as mybir
    import concourse.tile as tile
    from concourse import bacc, library_config
    from concourse.tile import add_dep_helper

    f32 = mybir.dt.float32
    bf16 = mybir.dt.bfloat16
    Exp = mybir.ActivationFunctionType.Exp
    Copy = mybir.ActivationFunctionType.Copy

    nc = bacc.Bacc(
        "TRN2",
        target_bir_lowering=False,
        debug=False,
        enable_asserts=True,
        num_devices=N_CORES,
    )

    xt_d = nc.dram_tensor("xt", [D_MODEL, S], bf16, kind="ExternalInput")
    w_d = {
        n: nc.dram_tensor(n, [D_MODEL, 128], bf16, kind="ExternalInput")
        for n in ("wq2", "wk2", "wv2")
    }
    wo_d = nc.dram_tensor("wo2", [64, 3 * D_MODEL], bf16, kind="ExternalInput")
    xqb_d = nc.dram_tensor("xqb", [D_MODEL, 2048], bf16, kind="ExternalInput")
    wqbs_d = nc.dram_tensor("wqbs", [D_MODEL, 64], bf16, kind="ExternalInput")
    cosqb_d = nc.dram_tensor("cosqb", [64, 2048], bf16, kind="ExternalInput")
    sinqb_d = nc.dram_tensor("sinqb", [64, 2048], bf16, kind="ExternalInput")
    maskh_d = nc.dram_tensor("maskh", [128, 1024], bf16, kind="ExternalInput")
    cosk_d = nc.dram_tensor("cosk", [128, S], bf16, kind="ExternalInput")
    sink_d = nc.dram_tensor("sink", [128, S], bf16, kind="ExternalInput")
    mask_d = nc.dram_tensor("maskab", [128, 512], bf16, kind="ExternalInput")
    out_d = nc.dram_tensor("out", [512, D_MODEL], f32, kind="ExternalOutput")
    o_part = nc.dram_tensor("o_part", [S, D_MODEL], bf16, kind="Internal")
    ors = [
        nc.dram_tensor(f"ors{j}", [(hi - lo) // 8, D_MODEL], bf16, kind="Internal")
        for j, (_, lo, hi) in enumerate(RS_CHUNKS)
    ]


    PAIRSWAP = [i ^ 1 for i in range(32)]

    with tile.TileContext(nc) as tc:
        with (
            tc.tile_pool(name="const", bufs=1) as cpool,
            tc.tile_pool(name="rope", bufs=4) as rpool,
            tc.tile_pool(name="expp", bufs=5) as epool,
            tc.tile_pool(name="norm", bufs=6) as npool,
            tc.tile_pool(name="rsrb", bufs=3) as rbpool,
            tc.tile_pool(name="ps", bufs=3, space="PSUM") as pspool,
            tc.tile_pool(name="ps_pv", bufs=2, space="PSUM") as psv,
        ):
            nc.gpsimd.load_library(library_config.attn)

            # ---- persistent tensors; first chunk's inputs loaded first ----
            def load_w(n):
                t = cpool.tile([128, EB * 128], bf16, tag=f"w_{n}", name=n)
                nc.sync.dma_start(
                    out=t[:].rearrange("p (e m) -> p e m", m=128),
                    in_=w_d[n][:].rearrange("(e p) m -> p e m", p=128),
                )
                return t

            def xt_load(lo, hi):
                nc.sync.dma_start(
                    out=xt_sb[:].rearrange("p (e s) -> p e s", s=S)[:, :, lo:hi],
                    in_=xt_d[:].rearrange("(e p) s -> p e s", p=128)[:, :, lo:hi],
                )

            wk_sb = load_w("wk2")
            xt_sb = cpool.tile([128, EB * S], bf16, tag="xt_sb")
            xt_load(0, 512)
            cosk_sb = cpool.tile([128, S], bf16, tag="cosk_sb")
            nc.sync.dma_start(out=cosk_sb[:, 0:512], in_=cosk_d[:, 0:512])
            sink_sb = cpool.tile([128, S], bf16, tag="sink_sb")
            nc.sync.dma_start(out=sink_sb[:, 0:512], in_=sink_d[:, 0:512])
            wq_sb = load_w("wq2")
            maskab = cpool.tile([128, 512], bf16, tag="maskab")
            nc.sync.dma_start(out=maskab[:], in_=mask_d[:])
            xqb_sb = cpool.tile([128, EB * 2048], bf16, tag="xqb_sb")
            nc.sync.dma_start(
                out=xqb_sb[:].rearrange("p (e s) -> p e s", s=2048)[:, :, 0:1024],
                in_=xqb_d[:].rearrange("(e p) s -> p e s", p=128)[:, :, 0:1024],
            )
            cosqb_sb = cpool.tile([128, 2048], bf16, tag="cosqb_sb")
            nc.sync.dma_start(out=cosqb_sb[64:128, :], in_=cosqb_d[:])
            sinqb_sb = cpool.tile([128, 2048], bf16, tag="sinqb_sb")
            nc.sync.dma_start(out=sinqb_sb[64:128, :], in_=sinqb_d[:])
            wqbs_sb = cpool.tile([128, EB * 64], bf16, tag="wqbs")
            nc.sync.dma_start(
                out=wqbs_sb[:].rearrange("p (e m) -> p e m", m=64),
                in_=wqbs_d[:].rearrange("(e p) m -> p e m", p=128),
            )
            maskh = cpool.tile([128, 1024], bf16, tag="maskh")
            nc.sync.dma_start(out=maskh[:], in_=maskh_d[:])
            wv_sb = load_w("wv2")
            wo_sb = cpool.tile([64, 3 * D_MODEL], bf16, tag="wo2")
            nc.sync.dma_start(out=wo_sb[:], in_=wo_d[:])
            xt_load(512, 1024)
            nc.sync.dma_start(out=cosk_sb[:, 512:S], in_=cosk_d[:, 512:S])
            nc.sync.dma_start(out=sink_sb[:, 512:S], in_=sink_d[:, 512:S])
            xt_load(1024, S)
            nc.sync.dma_start(
                out=xqb_sb[:].rearrange("p (e s) -> p e s", s=2048)[:, :, 1024:2048],
                in_=xqb_d[:].rearrange("(e p) s -> p e s", p=128)[:, :, 1024:2048],
            )

            qb = cpool.tile([128, 2048], bf16, tag="qb")
            attnb = cpool.tile([64, 2048], bf16, tag="attnb")
            ones64 = cpool.tile([65, 64], bf16, tag="ones64")
            nc.vector.memset(ones64[64:65, :], 1.0)
            qt = cpool.tile([128, S], bf16, tag="qt")
            ktc = [
                cpool.tile([128, 512], bf16, tag=f"kt{ch}", name=f"kt{ch}")
                for ch in range(N_CH)
            ]
            vc = [
                cpool.tile([128, 4 * VW], bf16, tag=f"va{ch}", name=f"va{ch}")
                for ch in range(N_CH)
            ]
            attn_sb = [cpool.tile([64, S], bf16, tag="attn0", name="attn0")]

            def rope(dst, src_ps, cos_ap, sin_ap):
                xb = rpool.tile([128, 512], bf16, tag="rope_x")
                nc.vector.tensor_copy(xb[:], src_ps[:])
                sh = rpool.tile([128, 512], bf16, tag="rope_sh")
                nc.vector.stream_shuffle(sh[:], xb[:], PAIRSWAP)
                nc.vector.tensor_mul(xb[:], xb[:], cos_ap)
                nc.vector.tensor_mul(sh[:], sh[:], sin_ap)
                nc.vector.tensor_add(dst, xb[:], sh[:])

            def attention_tile(slot, T):
                """Score/exp/PV matmuls for one (head-slot, 256-row q-tile).

                Software-pipelined: group g+1's score matmuls are issued
                before group g's PV matmuls so the PE never sits behind the
                exp. Normalization is issued later (see finalize_tile)."""
                ro = 64 * slot
                qslice = qt[ro:ro + 64, T * 256:T * 256 + 256]
                pv = psv.tile([65, 256], f32, tag="ps_pv")
                groups = [(pg, 2 if pg + 1 <= T else 1) for pg in range(0, T + 1, 2)]

                def issue_pv(pg, w, et):
                    for pi in range(w):
                        p = pg + pi
                        for j in range(2):
                            t = 2 * p + j
                            nc.tensor.matmul(
                                pv[:],
                                vc[t // 4][:, (t % 4) * VW + slot * 65:(t % 4) * VW + slot * 65 + 65],
                                et[:, (2 * pi + j) * 256:(2 * pi + j + 1) * 256],
                                start=(p == 0 and j == 0),
                                stop=(p == T and j == 1),
                            )

                prev = None
                for pg, w in groups:
                    sc = pspool.tile([128, 1024], f32, tag="ps")
                    for pi in range(w):
                        for j in range(2):
                            t = 2 * (pg + pi) + j
                            nc.tensor.matmul(
                                sc[:, (2 * pi + j) * 256:(2 * pi + j + 1) * 256],
                                ktc[t // 4][ro:ro + 64, (t % 4) * 128:(t % 4) * 128 + 128],
                                qslice,
                                start=True,
                                stop=True,
                            )
                    et = epool.tile([128, 1024], bf16, tag="et")
                    nc.scalar.activation(
                        et[:, 0:512 * w], sc[:, 0:512 * w], Exp, bias=0.0, scale=0.125
                    )
                    if pg + w - 1 == T:  # group holds the diagonal pair
                        off = 512 * (w - 1)
                        nc.vector.tensor_mul(
                            et[:, off:off + 512], et[:, off:off + 512], maskab[:]
                        )
                    if prev is not None:
                        issue_pv(*prev)
                    prev = (pg, w, et)
                issue_pv(*prev)
                # reduce + reciprocal now; broadcast and normalize deferred
                pvs = npool.tile([65, 256], f32, tag="pvs")
                nc.vector.tensor_copy(pvs[:], pv[:])
                rrow = npool.tile([65, 256], bf16, tag="rrow")
                with nc.allow_low_precision(reason="bf16 softmax denominators"):
                    nc.vector.reciprocal(rrow[64:65, :], pvs[64:65, :])
                return pvs, rrow

            def b_tile(m):
                """Head-B attention for the core's m-th q-tile (physical
                q-tile 2m+parity, supplied pre-permuted in qb). Runs 2m+2
                pairs; the parity-dependent [128,1024] maskh input handles
                the diagonal and the padded pair in the last group."""
                qsl = qb[64:128, m * 256:m * 256 + 256]
                pv = psv.tile([65, 256], f32, tag="ps_pv")
                npairs = 2 * m + 2

                def issue_pvb(pg, et):
                    for pi in range(2):
                        p = pg + pi
                        for j in range(2):
                            t = 2 * p + j
                            nc.tensor.matmul(
                                pv[:],
                                vc[t // 4][:, (t % 4) * VW + 65:(t % 4) * VW + 130],
                                et[:, (2 * pi + j) * 256:(2 * pi + j + 1) * 256],
                                start=(p == 0 and j == 0),
                                stop=(p == npairs - 1 and j == 1),
                            )

                prev = None
                for pg in range(0, npairs, 2):
                    sc = pspool.tile([128, 1024], f32, tag="ps")
                    for pi in range(2):
                        for j in range(2):
                            t = 2 * (pg + pi) + j
                            nc.tensor.matmul(
                                sc[:, (2 * pi + j) * 256:(2 * pi + j + 1) * 256],
                                ktc[t // 4][64:128, (t % 4) * 128:(t % 4) * 128 + 128],
                                qsl,
                                start=True,
                                stop=True,
                            )
                    et = epool.tile([128, 1024], bf16, tag="et")
                    nc.scalar.activation(
                        et[:], sc[:], Exp, bias=0.0, scale=0.125
                    )
                    if pg + 2 >= npairs:  # last group: diagonal + padding
                        nc.vector.tensor_mul(et[:], et[:], maskh[:])
                    if prev is not None:
                        issue_pvb(*prev)
                    prev = (pg, et)
                issue_pvb(*prev)
                pvs = npool.tile([65, 256], f32, tag="pvs")
                nc.vector.tensor_copy(pvs[:], pv[:])
                rrow = npool.tile([65, 256], bf16, tag="rrow")
                with nc.allow_low_precision(reason="bf16 softmax denominators"):
                    nc.vector.reciprocal(rrow[64:65, :], pvs[64:65, :])
                return pvs, rrow

            def normalize(pvs, rrow, dst):
                rb = psv.tile([65, 256], f32, tag="ps_pv")
                nc.tensor.matmul(
                    rb[0:64, :], ones64[64:65, :], rrow[64:65, :],
                    start=True, stop=True,
                )
                nc.vector.tensor_mul(dst, pvs[0:64, :], rb[0:64, :])

            def finalize_tile(T, handles):
                """Deferred normalize (broadcast via K=1 matmul) + output
                projection for q-tile T; issued one tile later so the
                reciprocal is ready and the PE never waits."""
                pvs, rrow = handles
                normalize(pvs, rrow, attn_sb[0][0:64, T * 256:T * 256 + 256])
                o_proj_pair(T)

            def finalize_btile(m, handles):
                pvs, rrow = handles
                normalize(pvs, rrow, attnb[0:64, m * 256:m * 256 + 256])

            def o_proj_pair(T):
                osb = rbpool.tile([128, 2 * D_MODEL], bf16, tag="osb")
                for half in range(2):
                    qtl = 2 * T + half
                    pot = pspool.tile([128, 1024], f32, tag="ps")
                    po = pot[:, 0:512]
                    po2 = pot[:, 512:768]
                    lhsT = attn_sb[0][0:64, qtl * 128:qtl * 128 + 128]
                    # head-B contribution rides the same accumulation: the
                    # weight slice is real or zero depending on whether this
                    # core's parity owns q-tile T (selected by host data)
                    wb = D_MODEL + (T % 2) * D_MODEL
                    lhsB = attnb[0:64, (T // 2) * 256 + (qtl % 2) * 128:(T // 2) * 256 + (qtl % 2) * 128 + 128]
                    nc.tensor.matmul(po, lhsT, wo_sb[:, 0:512], start=True, stop=False)
                    nc.tensor.matmul(
                        po, lhsB, wo_sb[:, wb:wb + 512], start=False, stop=True
                    )
                    nc.tensor.matmul(
                        po2, lhsT, wo_sb[:, 512:768], start=True, stop=False
                    )
                    nc.tensor.matmul(
                        po2, lhsB, wo_sb[:, wb + 512:wb + 768], start=False, stop=True
                    )
                    nc.vector.tensor_copy(
                        osb[:, half * D_MODEL:(half + 1) * D_MODEL],
                        pot[:, 0:768],
                    )
                last_opart[0] = nc.sync.dma_start(
                    out=o_part[T * 256:(T + 1) * 256, :].rearrange(
                        "(h p) d -> p h d", p=128
                    ),
                    in_=osb[:].rearrange("p (h d) -> p h d", d=D_MODEL),
                )

            def fire_rs(j):
                _, lo, hi = RS_CHUNKS[j]
                nc.gpsimd.collective_compute(
                    "ReduceScatter",
                    mybir.AluOpType.add,
                    replica_groups=[list(range(N_CORES))],
                    ins=[o_part[lo:hi, :]],
                    outs=[ors[j][:]],
                )

            def readback(j):
                _, lo, hi = RS_CHUNKS[j]
                shard = (hi - lo) // 8
                for b in range(0, shard, 128):
                    bb = min(128, shard - b)
                    rt = rbpool.tile([128, D_MODEL], bf16, tag="rt")
                    rd = nc.sync.dma_start(out=rt[0:bb, :], in_=ors[j][b:b + bb, :])
                    # keep readback DMAs after all o_part writes so the
                    # round-robin DMA-queue counts of collective waits never
                    # include collective-dependent transfers
                    if last_opart[0] is not None:
                        add_dep_helper(
                            rd.ins, last_opart[0].ins, sync=True,
                            reason="readback after o_part stream",
                        )
                    rtf = rbpool.tile([128, D_MODEL], f32, tag="rtf")
                    nc.vector.tensor_copy(rtf[0:bb, :], rt[0:bb, :])
                    nc.sync.dma_start(
                        out=out_d[RS_OUT_OFF[j] + b:RS_OUT_OFF[j] + b + bb, :],
                        in_=rtf[0:bb, :],
                    )

            # ---- main loop: projection chunk ch, then attention q-tiles;
            # tile finalization (normalize + o_proj + RS) runs one tile behind
            rs_next = 0
            last_opart = [None]
            b_done = False
            pending = None  # (kind, idx, handles)

            def do_proj_kq(ch):
                def xt_t_slice(eb, lo, hi):
                    return xt_sb[:, eb * S + ch * 512 + lo:eb * S + ch * 512 + hi]
                ck = cosk_sb[:, ch * 512:(ch + 1) * 512]
                sk = sink_sb[:, ch * 512:(ch + 1) * 512]

                # K^T and Q^T share one PSUM tile; RoPE applied to both
                psKQ = pspool.tile([128, 1024], f32, tag="ps")
                for eb in range(EB):
                    nc.tensor.matmul(
                        psKQ[:, 0:512],
                        wk_sb[:, eb * 128:(eb + 1) * 128],
                        xt_t_slice(eb, 0, 512),
                        start=(eb == 0),
                        stop=(eb == EB - 1),
                    )
                for eb in range(EB):
                    nc.tensor.matmul(
                        psKQ[:, 512:1024],
                        wq_sb[:, eb * 128:(eb + 1) * 128],
                        xt_t_slice(eb, 0, 512),
                        start=(eb == 0),
                        stop=(eb == EB - 1),
                    )
                rope(ktc[ch][:], psKQ[:, 0:512], ck, sk)
                rope(qt[:, ch * 512:(ch + 1) * 512], psKQ[:, 512:1024], ck, sk)

            def do_proj_v(ch):
                def xt_t_slice(eb, lo, hi):
                    return xt_sb[:, eb * S + ch * 512 + lo:eb * S + ch * 512 + hi]
                # V (natural layout, interleaved ones column per head):
                # 4 s-tiles accumulate into one PSUM tile
                psV4 = pspool.tile([128, 1024], f32, tag="ps")
                for stl in range(4):
                    for eb in range(EB):
                        nc.tensor.matmul(
                            psV4[:, stl * 256:stl * 256 + 128],
                            xt_t_slice(eb, stl * 128, stl * 128 + 128),
                            wv_sb[:, eb * 128:(eb + 1) * 128],
                            start=(eb == 0),
                            stop=(eb == EB - 1),
                        )
                for stl in range(4):
                    vtile = vc[ch][:, stl * VW:(stl + 1) * VW].rearrange(
                        "p (h d) -> p h d", d=65
                    )
                    nc.vector.memset(vtile[:, :, 64:65], 1.0)
                    nc.vector.tensor_copy(
                        vtile[:, :, 0:64],
                        psV4[:, stl * 256:stl * 256 + 128].rearrange(
                            "p (h d) -> p h d", d=64
                        ),
                    )

            def do_proj_qb(q2):
                """Project + RoPE a 512-col quarter of the permuted head-B
                q-columns into qb rows 64:128. The RoPE pair-swap comes from
                a second projection against host-swapped weights, so every
                vector op runs full-width and no partition shuffle is
                needed (rows 0:64 carry unused garbage)."""
                qs = slice(q2 * 512, q2 * 512 + 512)
                psB = pspool.tile([128, 1024], f32, tag="ps")
                for eb in range(EB):
                    nc.tensor.matmul(
                        psB[64:128, 0:512],
                        wq_sb[:, eb * 128 + 64:(eb + 1) * 128],
                        xqb_sb[:, eb * 2048 + q2 * 512:eb * 2048 + q2 * 512 + 512],
                        start=(eb == 0),
                        stop=(eb == EB - 1),
                    )
                for eb in range(EB):
                    nc.tensor.matmul(
                        psB[64:128, 512:1024],
                        wqbs_sb[:, eb * 64:(eb + 1) * 64],
                        xqb_sb[:, eb * 2048 + q2 * 512:eb * 2048 + q2 * 512 + 512],
                        start=(eb == 0),
                        stop=(eb == EB - 1),
                    )
                xb = rpool.tile([128, 512], bf16, tag="rope_x")
                nc.vector.tensor_copy(xb[64:128, :], psB[64:128, 0:512])
                sh = rpool.tile([128, 512], bf16, tag="rope_sh")
                nc.vector.tensor_copy(sh[64:128, :], psB[64:128, 512:1024])
                nc.vector.tensor_mul(xb[64:128, :], xb[64:128, :], cosqb_sb[64:128, qs])
                nc.vector.tensor_mul(sh[64:128, :], sh[64:128, :], sinqb_sb[64:128, qs])
                nc.vector.tensor_add(qb[64:128, qs], xb[64:128, :], sh[64:128, :])

            # projections run one chunk ahead, split and issued mid-chunk so
            # the attention tiles keep the Act engine fed at boundaries
            do_proj_kq(0)
            do_proj_v(0)
            do_proj_qb(0)
            do_proj_kq(1)
            do_proj_v(1)

            def pop_pending():
                nonlocal pending, rs_next, b_done
                if pending is None:
                    return
                kind, idx, ph = pending
                if kind == "A":
                    finalize_tile(idx, ph)
                    if (
                        with_rs
                        and rs_next < len(RS_CHUNKS)
                        and RS_CHUNKS[rs_next][0] == idx
                    ):
                        fire_rs(rs_next)
                        rs_next += 1
                else:
                    finalize_btile(idx, ph)
                pending = None

            for ch in range(N_CH):
                bh = b_tile(ch)
                pop_pending()
                pending = ("B", ch, bh)
                for T in (2 * ch, 2 * ch + 1):
                    handles = attention_tile(0, T)
                    pop_pending()
                    pending = ("A", T, handles)
                    if ch + 2 < N_CH:
                        if T == 2 * ch:
                            do_proj_kq(ch + 2)
                        else:
                            do_proj_v(ch + 2)
                    if ch in (1, 3, 5) and T == 2 * ch:
                        do_proj_qb((ch + 1) // 2)
            pop_pending()
            while with_rs and rs_next < len(RS_CHUNKS):
                fire_rs(rs_next)
                rs_next += 1
            if with_rs:
                for j in range(len(RS_CHUNKS)):
                    readback(j)


    nc.compile()
    return nc


_PROGRAM = None


def _get_program():
    global _PROGRAM
    if _PROGRAM is None:
        _PROGRAM = build_program()
    return _PROGRAM


def host_prep(in_features, token_positions, q_proj, k_proj, v_proj, o_proj):
    """Build the 8 per-core input maps."""
    x = np.asarray(in_features, np.float32).reshape(S, D_MODEL)
    tp = np.asarray(token_positions)
    qp = np.asarray(q_proj, np.float32)
    kp = np.asarray(k_proj, np.float32)
    vp = np.asarray(v_proj, np.float32)
    op = np.asarray(o_proj, np.float32)

    xt_bf = np.ascontiguousarray(x.T).astype(BF16)      # [768, 4096]
    wqT = np.ascontiguousarray(qp.T)                    # [in 768, out 768]
    wkT = np.ascontiguousarray(kp.T)
    wvT = np.ascontiguousarray(vp.T)
    opT = np.ascontiguousarray(op.T)                    # [in-dk 768, out 768]

    inv_freq = 1.0 / THETA ** (np.arange(0, DK, 2, dtype=np.float32) / DK)
    pos = np.clip(tp.astype(np.float32), 0, MAX_SEQ_LEN - 1)
    freq = pos[:, None] * inv_freq[None, :]             # [S, 32]
    cos_t, sin_t = np.cos(freq), np.sin(freq)

    r = np.arange(128)
    fidx = (r % 64) // 2
    sign = np.where(r % 2 == 0, -1.0, 1.0).astype(np.float32)
    cos128 = cos_t[:, fidx].T.astype(BF16)              # [128, S]
    sin128 = (sin_t[:, fidx].T * sign[:, None]).astype(BF16)

    ki = np.arange(128)[:, None]
    qi = np.arange(256)[None, :]
    mask_a = (ki <= qi).astype(np.float32)
    mask_b = (ki + 128 <= qi).astype(np.float32)
    maskab = np.concatenate([mask_a, mask_b], axis=1).astype(BF16)

    ones512 = np.ones((128, 512), np.float32)
    zeros512 = np.zeros((128, 512), np.float32)

    in_maps = []
    for c in range(N_CORES):
        hA = c
        hB = 8 + c // 2
        p = c % 2

        def wslice(wT):
            out = np.empty((D_MODEL, 128), np.float32)
            out[:, 0:64] = wT[:, hA * 64:(hA + 1) * 64]
            out[:, 64:128] = wT[:, hB * 64:(hB + 1) * 64]
            return out.astype(BF16)

        wo2 = np.zeros((64, 3 * D_MODEL), np.float32)
        wo2[:, 0:D_MODEL] = opT[hA * 64:(hA + 1) * 64, :]
        # slot 1 = used for even q-tiles, slot 2 = odd q-tiles; only the
        # parity this core owns carries real head-B weights
        wo2[:, (1 + p) * D_MODEL:(2 + p) * D_MODEL] = opT[hB * 64:(hB + 1) * 64, :]

        wqB = wqT[:, hB * 64:(hB + 1) * 64]
        swap = np.arange(64) ^ 1
        wqbs = np.ascontiguousarray(wqB[:, swap]).astype(BF16)

        qcolsB = np.concatenate(
            [np.arange(256 * (2 * m + p), 256 * (2 * m + p) + 256) for m in range(8)]
        )
        maskab_f = maskab.astype(np.float32)
        if p == 0:
            maskh = np.concatenate([maskab_f, zeros512], axis=1)
        else:
            maskh = np.concatenate([ones512, maskab_f], axis=1)

        in_maps.append(
            {
                "xt": xt_bf,
                "xqb": np.ascontiguousarray(xt_bf[:, qcolsB]),
                "wqbs": wqbs,
                "wq2": wslice(wqT),
                "wk2": wslice(wkT),
                "wv2": wslice(wvT),
                "wo2": wo2.astype(BF16),
                "cosk": cos128,
                "sink": sin128,
                "cosqb": np.ascontiguousarray(cos128[0:64, qcolsB]),
                "sinqb": np.ascontiguousarray(sin128[0:64, qcolsB]),
                "maskab": maskab,
                "maskh": maskh.astype(BF16),
            }
        )
    return in_maps


def assemble_output(results):
    out = np.empty((1, S, D_MODEL), np.float32)
    for c in range(N_CORES):
        r = np.asarray(results[c]["out"], np.float32)
        for j, (_, lo, hi) in enumerate(RS_CHUNKS):
            shard = (hi - lo) // 8
            oo = RS_OUT_OFF[j]
            out[0, lo + shard * c:lo + shard * (c + 1)] = r[oo:oo + shard]
    return out


def kernel(**inputs):
    from concourse.bass_utils import run_bass_kernel_spmd

    nc = _get_program()
    in_maps = host_prep(**inputs)
    res = run_bass_kernel_spmd(nc, in_maps, list(range(N_CORES)))
    return assemble_output(res.results)


if __name__ == "__main__":
    nc = build_program()
    print("program built and compiled")


# revision 8
# speedup vs baseline: 1.3822x; 1.0132x over previous
"""Trainium2 Bass kernel: causal MultiHeadAttention with RoPE (head-parallel).

B=1, S=4096, D=768, H=12 heads, dk=64, fp32 I/O. 8 NeuronCores, SPMD.

Sharding: head-parallel with split tail heads. Core c owns whole head
A=c (heads 0..7) plus HALF of head B=8+c//2: the q-tiles of parity c%2
(4 tiles of 256 rows each). Every core projects K/V/Q for its two heads
over the full sequence, runs full-causal attention for head A and its
four B q-tiles (identical instruction stream everywhere; the lone
parity-dependent bit is a [128,1024] mask input), computes bf16 partial
output projections, and combines them with ReduceScatter collectives:
an 8-way RS for the A-partials (q-sharded result) and a 4-way RS over
each parity group for the B-partials, which the host adds on top.
"""

import sys

if "/opt/trn_rl_repo" not in sys.path:
    sys.path.insert(0, "/opt/trn_rl_repo")

import numpy as np
import ml_dtypes

D_MODEL = 768
H = 12
DK = 64
S = 4096
THETA = 10000.0
MAX_SEQ_LEN = 4096
N_CORES = 8
EB = D_MODEL // 128   # 6 contraction blocks
N_CH = S // 512       # 8 sequence chunks
VW = 130              # V_aug row width per s-tile: 2 heads x (64+ones)

BF16 = ml_dtypes.bfloat16

# ReduceScatter chunks: fire after q-tile T_FIRE, covering q rows [lo, hi).
# Fired one q-tile after the covered rows complete so the collective's
# input-DMA waits are already satisfied and never block the Pool queue.
RS_CHUNKS = [
    (10, 0, 2560),
    (13, 2560, 3584),
    (15, 3584, 4096),
]
# out_d row offset of each chunk's shard
RS_OUT_OFF = [0, 320, 448]


def build_program(with_rs=True):
    import concourse.mybir as mybir
    import concourse.tile as tile
    from concourse import bacc, library_config
    from concourse.tile import add_dep_helper

    f32 = mybir.dt.float32
    bf16 = mybir.dt.bfloat16
    Exp = mybir.ActivationFunctionType.Exp
    Copy = mybir.ActivationFunctionType.Copy

    nc = bacc.Bacc(
        "TRN2",
        target_bir_lowering=False,
        debug=False,
        enable_asserts=True,
        num_devices=N_CORES,
    )

    xt_d = nc.dram_tensor("xt", [D_MODEL, S], bf16, kind="ExternalInput")
    w_d = {
        n: nc.dram_tensor(n, [D_MODEL, 128], bf16, kind="ExternalInput")
        for n in ("wq2", "wk2", "wv2")
    }
    wo_d = nc.dram_tensor("wo2", [64, 3 * D_MODEL], bf16, kind="ExternalInput")
    xqb_d = nc.dram_tensor("xqb", [D_MODEL, 2048], bf16, kind="ExternalInput")
    wqbs_d = nc.dram_tensor("wqbs", [D_MODEL, 64], bf16, kind="ExternalInput")
    cosqb_d = nc.dram_tensor("cosqb", [64, 2048], bf16, kind="ExternalInput")
    sinqb_d = nc.dram_tensor("sinqb", [64, 2048], bf16, kind="ExternalInput")
    maskh_d = nc.dram_tensor("maskh", [128, 1024], bf16, kind="ExternalInput")
    cosk_d = nc.dram_tensor("cosk", [128, S], bf16, kind="ExternalInput")
    sink_d = nc.dram_tensor("sink", [128, S], bf16, kind="ExternalInput")
    mask_d = nc.dram_tensor("maskab", [128, 512], bf16, kind="ExternalInput")
    out_d = nc.dram_tensor("out", [512, D_MODEL], f32, kind="ExternalOutput")
    o_part = nc.dram_tensor("o_part", [S, D_MODEL], bf16, kind="Internal")
    ors = [
        nc.dram_tensor(f"ors{j}", [(hi - lo) // 8, D_MODEL], bf16, kind="Internal")
        for j, (_, lo, hi) in enumerate(RS_CHUNKS)
    ]


    PAIRSWAP = [i ^ 1 for i in range(32)]

    with tile.TileContext(nc) as tc:
        with (
            tc.tile_pool(name="const", bufs=1) as cpool,
            tc.tile_pool(name="rope", bufs=4) as rpool,
            tc.tile_pool(name="expp", bufs=5) as epool,
            tc.tile_pool(name="norm", bufs=6) as npool,
            tc.tile_pool(name="rsrb", bufs=3) as rbpool,
            tc.tile_pool(name="ps", bufs=3, space="PSUM") as pspool,
            tc.tile_pool(name="ps_pv", bufs=2, space="PSUM") as psv,
        ):
            nc.gpsimd.load_library(library_config.attn)

            # ---- persistent tensors; first chunk's inputs loaded first ----
            def load_w(n):
                t = cpool.tile([128, EB * 128], bf16, tag=f"w_{n}", name=n)
                nc.sync.dma_start(
                    out=t[:].rearrange("p (e m) -> p e m", m=128),
                    in_=w_d[n][:].rearrange("(e p) m -> p e m", p=128),
                )
                return t

            def xt_load(lo, hi):
                nc.sync.dma_start(
                    out=xt_sb[:].rearrange("p (e s) -> p e s", s=S)[:, :, lo:hi],
                    in_=xt_d[:].rearrange("(e p) s -> p e s", p=128)[:, :, lo:hi],
                )

            wk_sb = load_w("wk2")
            xt_sb = cpool.tile([128, EB * S], bf16, tag="xt_sb")
            xt_load(0, 512)
            cosk_sb = cpool.tile([128, S], bf16, tag="cosk_sb")
            nc.sync.dma_start(out=cosk_sb[:, 0:512], in_=cosk_d[:, 0:512])
            sink_sb = cpool.tile([128, S], bf16, tag="sink_sb")
            nc.sync.dma_start(out=sink_sb[:, 0:512], in_=sink_d[:, 0:512])
            wq_sb = load_w("wq2")
            maskab = cpool.tile([128, 512], bf16, tag="maskab")
            nc.sync.dma_start(out=maskab[:], in_=mask_d[:])
            xqb_sb = cpool.tile([128, EB * 2048], bf16, tag="xqb_sb")
            nc.sync.dma_start(
                out=xqb_sb[:].rearrange("p (e s) -> p e s", s=2048)[:, :, 0:1024],
                in_=xqb_d[:].rearrange("(e p) s -> p e s", p=128)[:, :, 0:1024],
            )
            cosqb_sb = cpool.tile([128, 2048], bf16, tag="cosqb_sb")
            nc.sync.dma_start(out=cosqb_sb[64:128, :], in_=cosqb_d[:])
            sinqb_sb = cpool.tile([128, 2048], bf16, tag="sinqb_sb")
            nc.sync.dma_start(out=sinqb_sb[64:128, :], in_=sinqb_d[:])
            wqbs_sb = cpool.tile([128, EB * 64], bf16, tag="wqbs")
            nc.sync.dma_start(
                out=wqbs_sb[:].rearrange("p (e m) -> p e m", m=64),
                in_=wqbs_d[:].rearrange("(e p) m -> p e m", p=128),
            )
            maskh = cpool.tile([128, 1024], bf16, tag="maskh")
            nc.sync.dma_start(out=maskh[:], in_=maskh_d[:])
            wv_sb = load_w("wv2")
            wo_sb = cpool.tile([64, 3 * D_MODEL], bf16, tag="wo2")
            nc.sync.dma_start(out=wo_sb[:], in_=wo_d[:])
            xt_load(512, 1024)
            nc.sync.dma_start(out=cosk_sb[:, 512:S], in_=cosk_d[:, 512:S])
            nc.sync.dma_start(out=sink_sb[:, 512:S], in_=sink_d[:, 512:S])
            xt_load(1024, S)
            nc.sync.dma_start(
                out=xqb_sb[:].rearrange("p (e s) -> p e s", s=2048)[:, :, 1024:2048],
                in_=xqb_d[:].rearrange("(e p) s -> p e s", p=128)[:, :, 1024:2048],
            )

            qb = cpool.tile([128, 2048], bf16, tag="qb")
            attnb = cpool.tile([64, 2048], bf16, tag="attnb")
            ones64 = cpool.tile([65, 64], bf16, tag="ones64")
            nc.vector.memset(ones64[64:65, :], 1.0)
            qt = cpool.tile([128, S], bf16, tag="qt")
            ktc = [
                cpool.tile([128, 512], bf16, tag=f"kt{ch}", name=f"kt{ch}")
                for ch in range(N_CH)
            ]
            vc = [
                cpool.tile([128, 4 * VW], bf16, tag=f"va{ch}", name=f"va{ch}")
                for ch in range(N_CH)
            ]
            attn_sb = [cpool.tile([64, S], bf16, tag="attn0", name="attn0")]

            def rope(dst, src_ps, cos_ap, sin_ap):
                xb = rpool.tile([128, 512], bf16, tag="rope_x")
                nc.vector.tensor_copy(xb[:], src_ps[:])
                sh = rpool.tile([128, 512], bf16, tag="rope_sh")
                nc.vector.stream_shuffle(sh[:], xb[:], PAIRSWAP)
                nc.vector.tensor_mul(xb[:], xb[:], cos_ap)
                nc.vector.tensor_mul(sh[:], sh[:], sin_ap)
                nc.vector.tensor_add(dst, xb[:], sh[:])

            def attention_tile(slot, T):
                """Score/exp/PV matmuls for one (head-slot, 256-row q-tile).

                Software-pipelined: group g+1's score matmuls are issued
                before group g's PV matmuls so the PE never sits behind the
                exp. Normalization is issued later (see finalize_tile)."""
                ro = 64 * slot
                qslice = qt[ro:ro + 64, T * 256:T * 256 + 256]
                pv = psv.tile([65, 256], f32, tag="ps_pv")
                groups = [(pg, 2 if pg + 1 <= T else 1) for pg in range(0, T + 1, 2)]

                def issue_pv(pg, w, et):
                    for pi in range(w):
                        p = pg + pi
                        for j in range(2):
                            t = 2 * p + j
                            nc.tensor.matmul(
                                pv[:],
                                vc[t // 4][:, (t % 4) * VW + slot * 65:(t % 4) * VW + slot * 65 + 65],
                                et[:, (2 * pi + j) * 256:(2 * pi + j + 1) * 256],
                                start=(p == 0 and j == 0),
                                stop=(p == T and j == 1),
                            )

                prev = None
                for pg, w in groups:
                    sc = pspool.tile([128, 1024], f32, tag="ps")
                    for pi in range(w):
                        for j in range(2):
                            t = 2 * (pg + pi) + j
                            nc.tensor.matmul(
                                sc[:, (2 * pi + j) * 256:(2 * pi + j + 1) * 256],
                                ktc[t // 4][ro:ro + 64, (t % 4) * 128:(t % 4) * 128 + 128],
                                qslice,
                                start=True,
                                stop=True,
                            )
                    et = epool.tile([128, 1024], bf16, tag="et")
                    nc.scalar.activation(
                        et[:, 0:512 * w], sc[:, 0:512 * w], Exp, bias=0.0, scale=0.125
                    )
                    if pg + w - 1 == T:  # group holds the diagonal pair
                        off = 512 * (w - 1)
                        nc.vector.tensor_mul(
                            et[:, off:off + 512], et[:, off:off + 512], maskab[:]
                        )
                    if prev is not None:
                        issue_pv(*prev)
                    prev = (pg, w, et)
                issue_pv(*prev)
                # reduce + reciprocal now; broadcast and normalize deferred
                pvs = npool.tile([65, 256], f32, tag="pvs")
                nc.vector.tensor_copy(pvs[:], pv[:])
                rrow = npool.tile([65, 256], bf16, tag="rrow")
                with nc.allow_low_precision(reason="bf16 softmax denominators"):
                    nc.vector.reciprocal(rrow[64:65, :], pvs[64:65, :])
                return pvs, rrow

            def b_tile(m):
                """Head-B attention for the core's m-th q-tile (physical
                q-tile 2m+parity, supplied pre-permuted in qb). Runs 2m+2
                pairs; the parity-dependent [128,1024] maskh input handles
                the diagonal and the padded pair in the last group."""
                qsl = qb[64:128, m * 256:m * 256 + 256]
                pv = psv.tile([65, 256], f32, tag="ps_pv")
                npairs = 2 * m + 2

                def issue_pvb(pg, et):
                    for pi in range(2):
                        p = pg + pi
                        for j in range(2):
                            t = 2 * p + j
                            nc.tensor.matmul(
                                pv[:],
                                vc[t // 4][:, (t % 4) * VW + 65:(t % 4) * VW + 130],
                                et[:, (2 * pi + j) * 256:(2 * pi + j + 1) * 256],
                                start=(p == 0 and j == 0),
                                stop=(p == npairs - 1 and j == 1),
                            )

                prev = None
                for pg in range(0, npairs, 2):
                    sc = pspool.tile([128, 1024], f32, tag="ps")
                    for pi in range(2):
                        for j in range(2):
                            t = 2 * (pg + pi) + j
                            nc.tensor.matmul(
                                sc[:, (2 * pi + j) * 256:(2 * pi + j + 1) * 256],
                                ktc[t // 4][64:128, (t % 4) * 128:(t % 4) * 128 + 128],
                                qsl,
                                start=True,
                                stop=True,
                            )
                    et = epool.tile([128, 1024], bf16, tag="et")
                    nc.scalar.activation(
                        et[:], sc[:], Exp, bias=0.0, scale=0.125
                    )
                    if pg + 2 >= npairs:  # last group: diagonal + padding
                        nc.vector.tensor_mul(et[:], et[:], maskh[:])
                    if prev is not None:
                        issue_pvb(*prev)
                    prev = (pg, et)
                issue_pvb(*prev)
                pvs = npool.tile([65, 256], f32, tag="pvs")
                nc.vector.tensor_copy(pvs[:], pv[:])
                rrow = npool.tile([65, 256], bf16, tag="rrow")
                with nc.allow_low_precision(reason="bf16 softmax denominators"):
                    nc.vector.reciprocal(rrow[64:65, :], pvs[64:65, :])
                return pvs, rrow

            def normalize(pvs, rrow, dst):
                rb = psv.tile([65, 256], f32, tag="ps_pv")
                nc.tensor.matmul(
                    rb[0:64, :], ones64[64:65, :], rrow[64:65, :],
                    start=True, stop=True,
                )
                nc.vector.tensor_mul(dst, pvs[0:64, :], rb[0:64, :])

            def finalize_tile(T, handles):
                """Deferred normalize (broadcast via K=1 matmul) + output
                projection for q-tile T; issued one tile later so the
                reciprocal is ready and the PE never waits."""
                pvs, rrow = handles
                normalize(pvs, rrow, attn_sb[0][0:64, T * 256:T * 256 + 256])
                o_proj_pair(T)

            def finalize_btile(m, handles):
                pvs, rrow = handles
                normalize(pvs, rrow, attnb[0:64, m * 256:m * 256 + 256])

            def o_proj_pair(T):
                osb = rbpool.tile([128, 2 * D_MODEL], bf16, tag="osb")
                for half in range(2):
                    qtl = 2 * T + half
                    pot = pspool.tile([128, 1024], f32, tag="ps")
                    po = pot[:, 0:512]
                    po2 = pot[:, 512:768]
                    lhsT = attn_sb[0][0:64, qtl * 128:qtl * 128 + 128]
                    # head-B contribution rides the same accumulation: the
                    # weight slice is real or zero depending on whether this
                    # core's parity owns q-tile T (selected by host data)
                    wb = D_MODEL + (T % 2) * D_MODEL
                    lhsB = attnb[0:64, (T // 2) * 256 + (qtl % 2) * 128:(T // 2) * 256 + (qtl % 2) * 128 + 128]
                    nc.tensor.matmul(po, lhsT, wo_sb[:, 0:512], start=True, stop=False)
                    nc.tensor.matmul(
                        po, lhsB, wo_sb[:, wb:wb + 512], start=False, stop=True
                    )
                    nc.tensor.matmul(
                        po2, lhsT, wo_sb[:, 512:768], start=True, stop=False
                    )
                    nc.tensor.matmul(
                        po2, lhsB, wo_sb[:, wb + 512:wb + 768], start=False, stop=True
                    )
                    nc.vector.tensor_copy(
                        osb[:, half * D_MODEL:(half + 1) * D_MODEL],
                        pot[:, 0:768],
                    )
                last_opart[0] = nc.sync.dma_start(
                    out=o_part[T * 256:(T + 1) * 256, :].rearrange(
                        "(h p) d -> p h d", p=128
                    ),
                    in_=osb[:].rearrange("p (h d) -> p h d", d=D_MODEL),
                )

            def fire_rs(j):
                _, lo, hi = RS_CHUNKS[j]
                nc.gpsimd.collective_compute(
                    "ReduceScatter",
                    mybir.AluOpType.add,
                    replica_groups=[list(range(N_CORES))],
                    ins=[o_part[lo:hi, :]],
                    outs=[ors[j][:]],
                )

            def readback(j):
                _, lo, hi = RS_CHUNKS[j]
                shard = (hi - lo) // 8
                for b in range(0, shard, 128):
                    bb = min(128, shard - b)
                    rt = rbpool.tile([128, D_MODEL], bf16, tag="rt")
                    rd = nc.sync.dma_start(out=rt[0:bb, :], in_=ors[j][b:b + bb, :])
                    # keep readback DMAs after all o_part writes so the
                    # round-robin DMA-queue counts of collective waits never
                    # include collective-dependent transfers
                    if last_opart[0] is not None:
                        add_dep_helper(
                            rd.ins, last_opart[0].ins, sync=True,
                            reason="readback after o_part stream",
                        )
                    rtf = rbpool.tile([128, D_MODEL], f32, tag="rtf")
                    nc.vector.tensor_copy(rtf[0:bb, :], rt[0:bb, :])
                    nc.sync.dma_start(
                        out=out_d[RS_OUT_OFF[j] + b:RS_OUT_OFF[j] + b + bb, :],
                        in_=rtf[0:bb, :],
                    )

            # ---- main loop: projection chunk ch, then attention q-tiles;
            # tile finalization (normalize + o_proj + RS) runs one tile behind
            rs_next = 0
            last_opart = [None]
            b_done = False
            pending = None  # (kind, idx, handles)

            def do_proj_kq(ch):
                def xt_t_slice(eb, lo, hi):
                    return xt_sb[:, eb * S + ch * 512 + lo:eb * S + ch * 512 + hi]
                ck = cosk_sb[:, ch * 512:(ch + 1) * 512]
                sk = sink_sb[:, ch * 512:(ch + 1) * 512]

                # K^T and Q^T share one PSUM tile; RoPE applied to both
                psKQ = pspool.tile([128, 1024], f32, tag="ps")
                for eb in range(EB):
                    nc.tensor.matmul(
                        psKQ[:, 0:512],
                        wk_sb[:, eb * 128:(eb + 1) * 128],
                        xt_t_slice(eb, 0, 512),
                        start=(eb == 0),
                        stop=(eb == EB - 1),
                    )
                for eb in range(EB):
                    nc.tensor.matmul(
                        psKQ[:, 512:1024],
                        wq_sb[:, eb * 128:(eb + 1) * 128],
                        xt_t_slice(eb, 0, 512),
                        start=(eb == 0),
                        stop=(eb == EB - 1),
                    )
                rope(ktc[ch][:], psKQ[:, 0:512], ck, sk)
                rope(qt[:, ch * 512:(ch + 1) * 512], psKQ[:, 512:1024], ck, sk)

            def do_proj_v(ch):
                def xt_t_slice(eb, lo, hi):
                    return xt_sb[:, eb * S + ch * 512 + lo:eb * S + ch * 512 + hi]
                # V (natural layout, interleaved ones column per head):
                # 4 s-tiles accumulate into one PSUM tile
                psV4 = pspool.tile([128, 1024], f32, tag="ps")
                for stl in range(4):
                    for eb in range(EB):
                        nc.tensor.matmul(
                            psV4[:, stl * 256:stl * 256 + 128],
                            xt_t_slice(eb, stl * 128, stl * 128 + 128),
                            wv_sb[:, eb * 128:(eb + 1) * 128],
                            start=(eb == 0),
                            stop=(eb == EB - 1),
                        )
                for stl in range(4):
                    vtile = vc[ch][:, stl * VW:(stl + 1) * VW].rearrange(
                        "p (h d) -> p h d", d=65
                    )
                    nc.vector.memset(vtile[:, :, 64:65], 1.0)
                    nc.vector.tensor_copy(
                        vtile[:, :, 0:64],
                        psV4[:, stl * 256:stl * 256 + 128].rearrange(
                            "p (h d) -> p h d", d=64
                        ),
                    )

            def do_proj_qb(q2):
                """Project + RoPE a 512-col quarter of the permuted head-B
                q-columns into qb rows 64:128. The RoPE pair-swap comes from
                a second projection against host-swapped weights, so every
                vector op runs full-width and no partition shuffle is
                needed (rows 0:64 carry unused garbage)."""
                qs = slice(q2 * 512, q2 * 512 + 512)
                psB = pspool.tile([128, 1024], f32, tag="ps")
                for eb in range(EB):
                    nc.tensor.matmul(
                        psB[64:128, 0:512],
                        wq_sb[:, eb * 128 + 64:(eb + 1) * 128],
                        xqb_sb[:, eb * 2048 + q2 * 512:eb * 2048 + q2 * 512 + 512],
                        start=(eb == 0),
                        stop=(eb == EB - 1),
                    )
                for eb in range(EB):
                    nc.tensor.matmul(
                        psB[64:128, 512:1024],
                        wqbs_sb[:, eb * 64:(eb + 1) * 64],
                        xqb_sb[:, eb * 2048 + q2 * 512:eb * 2048 + q2 * 512 + 512],
                        start=(eb == 0),
                        stop=(eb == EB - 1),
                    )
                xb = rpool.tile([128, 512], bf16, tag="rope_x")
                nc.vector.tensor_copy(xb[64:128, :], psB[64:128, 0:512])
                sh = rpool.tile([128, 512], bf16, tag="rope_sh")
                nc.vector.tensor_copy(sh[64:128, :], psB[64:128, 512:1024])
                nc.vector.tensor_mul(xb[64:128, :], xb[64:128, :], cosqb_sb[64:128, qs])
                nc.vector.tensor_mul(sh[64:128, :], sh[64:128, :], sinqb_sb[64:128, qs])
                nc.vector.tensor_add(qb[64:128, qs], xb[64:128, :], sh[64:128, :])

            # projections run one chunk ahead, split and issued mid-chunk so
            # the attention tiles keep the Act engine fed at boundaries
            do_proj_kq(0)
            do_proj_v(0)
            do_proj_qb(0)
            do_proj_kq(1)
            do_proj_v(1)

            def pop_pending():
                nonlocal pending, rs_next, b_done
                if pending is None:
                    return
                kind, idx, ph = pending
                if kind == "A":
                    finalize_tile(idx, ph)
                    if (
                        with_rs
                        and rs_next < len(RS_CHUNKS)
                        and RS_CHUNKS[rs_next][0] == idx
                    ):
                        fire_rs(rs_next)
                        rs_next += 1
                else:
                    finalize_btile(idx, ph)
                pending = None

            for ch in range(N_CH):
                bh = b_tile(ch)
                pop_pending()
                pending = ("B", ch, bh)
                for T in (2 * ch, 2 * ch + 1):
                    handles = attention_tile(0, T)
                    pop_pending()
                    pending = ("A", T, handles)
                    if ch + 2 < N_CH:
                        if T == 2 * ch:
                            do_proj_kq(ch + 2)
                        else:
                            do_proj_v(ch + 2)
                    if ch in (1, 3, 5) and T == 2 * ch:
                        do_proj_qb((ch + 1) // 2)
            pop_pending()
            while with_rs and rs_next < len(RS_CHUNKS):
                fire_rs(rs_next)
                rs_next += 1
            if with_rs:
                for j in range(len(RS_CHUNKS)):
                    readback(j)


    nc.compile()
    return nc


_PROGRAM = None


def _get_program():
    global _PROGRAM
    if _PROGRAM is None:
        _PROGRAM = build_program()
    return _PROGRAM


def host_prep(in_features, token_positions, q_proj, k_proj, v_proj, o_proj):
    """Build the 8 per-core input maps."""
    x = np.asarray(in_features, np.float32).reshape(S, D_MODEL)
    tp = np.asarray(token_positions)
    qp = np.asarray(q_proj, np.float32)
    kp = np.asarray(k_proj, np.float32)
    vp = np.asarray(v_proj, np.float32)
    op = np.asarray(o_proj, np.float32)

    xt_bf = np.ascontiguousarray(x.T).astype(BF16)      # [768, 4096]
    wqT = np.ascontiguousarray(qp.T)                    # [in 768, out 768]
    wkT = np.ascontiguousarray(kp.T)
    wvT = np.ascontiguousarray(vp.T)
    opT = np.ascontiguousarray(op.T)                    # [in-dk 768, out 768]

    inv_freq = 1.0 / THETA ** (np.arange(0, DK, 2, dtype=np.float32) / DK)
    pos = np.clip(tp.astype(np.float32), 0, MAX_SEQ_LEN - 1)
    freq = pos[:, None] * inv_freq[None, :]             # [S, 32]
    cos_t, sin_t = np.cos(freq), np.sin(freq)

    r = np.arange(128)
    fidx = (r % 64) // 2
    sign = np.where(r % 2 == 0, -1.0, 1.0).astype(np.float32)
    cos128 = cos_t[:, fidx].T.astype(BF16)              # [128, S]
    sin128 = (sin_t[:, fidx].T * sign[:, None]).astype(BF16)

    ki = np.arange(128)[:, None]
    qi = np.arange(256)[None, :]
    mask_a = (ki <= qi).astype(np.float32)
    mask_b = (ki + 128 <= qi).astype(np.float32)
    maskab = np.concatenate([mask_a, mask_b], axis=1).astype(BF16)

    ones512 = np.ones((128, 512), np.float32)
    zeros512 = np.zeros((128, 512), np.float32)

    in_maps = []
    for c in range(N_CORES):
        hA = c
        hB = 8 + c // 2
        p = c % 2

        def wslice(wT):
            out = np.empty((D_MODEL, 128), np.float32)
            out[:, 0:64] = wT[:, hA * 64:(hA + 1) * 64]
            out[:, 64:128] = wT[:, hB * 64:(hB + 1) * 64]
            return out.astype(BF16)

        wo2 = np.zeros((64, 3 * D_MODEL), np.float32)
        wo2[:, 0:D_MODEL] = opT[hA * 64:(hA + 1) * 64, :]
        # slot 1 = used for even q-tiles, slot 2 = odd q-tiles; only the
        # parity this core owns carries real head-B weights
        wo2[:, (1 + p) * D_MODEL:(2 + p) * D_MODEL] = opT[hB * 64:(hB + 1) * 64, :]

        wqB = wqT[:, hB * 64:(hB + 1) * 64]
        swap = np.arange(64) ^ 1
        wqbs = np.ascontiguousarray(wqB[:, swap]).astype(BF16)

        qcolsB = np.concatenate(
            [np.arange(256 * (2 * m + p), 256 * (2 * m + p) + 256) for m in range(8)]
        )
        maskab_f = maskab.astype(np.float32)
        if p == 0:
            maskh = np.concatenate([maskab_f, zeros512], axis=1)
        else:
            maskh = np.concatenate([ones512, maskab_f], axis=1)

        in_maps.append(
            {
                "xt": xt_bf,
                "xqb": np.ascontiguousarray(xt_bf[:, qcolsB]),
                "wqbs": wqbs,
                "wq2": wslice(wqT),
                "wk2": wslice(wkT),
                "wv2": wslice(wvT),
                "wo2": wo2.astype(BF16),
                "cosk": cos128,
                "sink": sin128,
                "cosqb": np.ascontiguousarray(cos128[0:64, qcolsB]),
                "sinqb": np.ascontiguousarray(sin128[0:64, qcolsB]),
                "maskab": maskab,
                "maskh": maskh.astype(BF16),
            }
        )
    return in_maps


def assemble_output(results):
    out = np.empty((1, S, D_MODEL), np.float32)
    for c in range(N_CORES):
        r = np.asarray(results[c]["out"], np.float32)
        for j, (_, lo, hi) in enumerate(RS_CHUNKS):
            shard = (hi - lo) // 8
            oo = RS_OUT_OFF[j]
            out[0, lo + shard * c:lo + shard * (c + 1)] = r[oo:oo + shard]
    return out


def kernel(**inputs):
    from concourse.bass_utils import run_bass_kernel_spmd

    nc = _get_program()
    in_maps = host_prep(**inputs)
    res = run_bass_kernel_spmd(nc, in_maps, list(range(N_CORES)))
    return assemble_output(res.results)


if __name__ == "__main__":
    nc = build_program()
    print("program built and compiled")


# revision 12
# speedup vs baseline: 1.3830x; 1.0006x over previous
"""Trainium2 Bass kernel: causal MultiHeadAttention with RoPE (head-parallel).

B=1, S=4096, D=768, H=12 heads, dk=64, fp32 I/O. 8 NeuronCores, SPMD.

Sharding: head-parallel with split tail heads. Core c owns whole head
A=c (heads 0..7) plus HALF of head B=8+c//2: the q-tiles of parity c%2
(4 tiles of 256 rows each). Every core projects K/V/Q for its two heads
over the full sequence, runs full-causal attention for head A and its
four B q-tiles (identical instruction stream everywhere; the lone
parity-dependent bit is a [128,1024] mask input), computes bf16 partial
output projections, and combines them with ReduceScatter collectives:
an 8-way RS for the A-partials (q-sharded result) and a 4-way RS over
each parity group for the B-partials, which the host adds on top.
"""

import sys

if "/opt/trn_rl_repo" not in sys.path:
    sys.path.insert(0, "/opt/trn_rl_repo")

import numpy as np
import ml_dtypes

D_MODEL = 768
H = 12
DK = 64
S = 4096
THETA = 10000.0
MAX_SEQ_LEN = 4096
N_CORES = 8
EB = D_MODEL // 128   # 6 contraction blocks
N_CH = S // 512       # 8 sequence chunks
VW = 130              # V_aug row width per s-tile: 2 heads x (64+ones)

BF16 = ml_dtypes.bfloat16

# ReduceScatter chunks: fire after q-tile T_FIRE, covering q rows [lo, hi).
# Fired one q-tile after the covered rows complete so the collective's
# input-DMA waits are already satisfied and never block the Pool queue.
RS_CHUNKS = [
    (10, 0, 2560),
    (13, 2560, 3584),
    (15, 3584, 4096),
]
# out_d row offset of each chunk's shard
RS_OUT_OFF = [0, 320, 448]


def build_program(with_rs=True):
    import concourse.mybir as mybir
    import concourse.tile as tile
    from concourse import bacc, library_config
    from concourse.tile import add_dep_helper

    f32 = mybir.dt.float32
    bf16 = mybir.dt.bfloat16
    Exp = mybir.ActivationFunctionType.Exp
    Copy = mybir.ActivationFunctionType.Copy

    nc = bacc.Bacc(
        "TRN2",
        target_bir_lowering=False,
        debug=False,
        enable_asserts=True,
        num_devices=N_CORES,
    )

    xt_d = nc.dram_tensor("xt", [D_MODEL, S], bf16, kind="ExternalInput")
    w_d = {
        n: nc.dram_tensor(n, [D_MODEL, 128], bf16, kind="ExternalInput")
        for n in ("wq2", "wk2", "wv2")
    }
    wo_d = nc.dram_tensor("wo2", [64, 3 * D_MODEL], bf16, kind="ExternalInput")
    xqb_d = nc.dram_tensor("xqb", [D_MODEL, 2048], bf16, kind="ExternalInput")
    wqbs_d = nc.dram_tensor("wqbs", [D_MODEL, 64], bf16, kind="ExternalInput")
    cosqb_d = nc.dram_tensor("cosqb", [64, 2048], bf16, kind="ExternalInput")
    sinqb_d = nc.dram_tensor("sinqb", [64, 2048], bf16, kind="ExternalInput")
    maskh_d = nc.dram_tensor("maskh", [128, 1024], bf16, kind="ExternalInput")
    cosk_d = nc.dram_tensor("cosk", [128, S], bf16, kind="ExternalInput")
    sink_d = nc.dram_tensor("sink", [128, S], bf16, kind="ExternalInput")
    mask_d = nc.dram_tensor("maskab", [128, 512], bf16, kind="ExternalInput")
    out_d = nc.dram_tensor("out", [512, D_MODEL], f32, kind="ExternalOutput")
    o_part = nc.dram_tensor("o_part", [S, D_MODEL], bf16, kind="Internal")
    ors = [
        nc.dram_tensor(f"ors{j}", [(hi - lo) // 8, D_MODEL], bf16, kind="Internal")
        for j, (_, lo, hi) in enumerate(RS_CHUNKS)
    ]


    PAIRSWAP = [i ^ 1 for i in range(32)]

    with tile.TileContext(nc) as tc:
        with (
            tc.tile_pool(name="const", bufs=1) as cpool,
            tc.tile_pool(name="rope", bufs=4) as rpool,
            tc.tile_pool(name="expp", bufs=6) as epool,
            tc.tile_pool(name="norm", bufs=6) as npool,
            tc.tile_pool(name="rsrb", bufs=2) as rbpool,
            tc.tile_pool(name="ps", bufs=3, space="PSUM") as pspool,
            tc.tile_pool(name="ps_pv", bufs=2, space="PSUM") as psv,
        ):
            nc.gpsimd.load_library(library_config.attn)

            # ---- persistent tensors; first chunk's inputs loaded first ----
            def load_w(n):
                t = cpool.tile([128, EB * 128], bf16, tag=f"w_{n}", name=n)
                nc.sync.dma_start(
                    out=t[:].rearrange("p (e m) -> p e m", m=128),
                    in_=w_d[n][:].rearrange("(e p) m -> p e m", p=128),
                )
                return t

            def xt_load(lo, hi):
                nc.sync.dma_start(
                    out=xt_sb[:].rearrange("p (e s) -> p e s", s=S)[:, :, lo:hi],
                    in_=xt_d[:].rearrange("(e p) s -> p e s", p=128)[:, :, lo:hi],
                )

            wk_sb = load_w("wk2")
            xt_sb = cpool.tile([128, EB * S], bf16, tag="xt_sb")
            xt_load(0, 512)
            cosk_sb = cpool.tile([128, S], bf16, tag="cosk_sb")
            nc.sync.dma_start(out=cosk_sb[:, 0:512], in_=cosk_d[:, 0:512])
            sink_sb = cpool.tile([128, S], bf16, tag="sink_sb")
            nc.sync.dma_start(out=sink_sb[:, 0:512], in_=sink_d[:, 0:512])
            wq_sb = load_w("wq2")
            maskab = cpool.tile([128, 512], bf16, tag="maskab")
            nc.sync.dma_start(out=maskab[:], in_=mask_d[:])
            xqb_sb = cpool.tile([128, EB * 2048], bf16, tag="xqb_sb")
            nc.sync.dma_start(
                out=xqb_sb[:].rearrange("p (e s) -> p e s", s=2048)[:, :, 0:1024],
                in_=xqb_d[:].rearrange("(e p) s -> p e s", p=128)[:, :, 0:1024],
            )
            cosqb_sb = cpool.tile([128, 2048], bf16, tag="cosqb_sb")
            nc.sync.dma_start(out=cosqb_sb[64:128, :], in_=cosqb_d[:])
            sinqb_sb = cpool.tile([128, 2048], bf16, tag="sinqb_sb")
            nc.sync.dma_start(out=sinqb_sb[64:128, :], in_=sinqb_d[:])
            wqbs_sb = cpool.tile([128, EB * 64], bf16, tag="wqbs")
            nc.sync.dma_start(
                out=wqbs_sb[:].rearrange("p (e m) -> p e m", m=64),
                in_=wqbs_d[:].rearrange("(e p) m -> p e m", p=128),
            )
            maskh = cpool.tile([128, 1024], bf16, tag="maskh")
            nc.sync.dma_start(out=maskh[:], in_=maskh_d[:])
            wv_sb = load_w("wv2")
            wo_sb = cpool.tile([64, 3 * D_MODEL], bf16, tag="wo2")
            nc.sync.dma_start(out=wo_sb[:], in_=wo_d[:])
            xt_load(512, 1024)
            nc.sync.dma_start(out=cosk_sb[:, 512:S], in_=cosk_d[:, 512:S])
            nc.sync.dma_start(out=sink_sb[:, 512:S], in_=sink_d[:, 512:S])
            xt_load(1024, S)
            nc.sync.dma_start(
                out=xqb_sb[:].rearrange("p (e s) -> p e s", s=2048)[:, :, 1024:2048],
                in_=xqb_d[:].rearrange("(e p) s -> p e s", p=128)[:, :, 1024:2048],
            )

            qb = cpool.tile([128, 2048], bf16, tag="qb")
            attnb = cpool.tile([64, 2048], bf16, tag="attnb")
            ones64 = cpool.tile([65, 64], bf16, tag="ones64")
            nc.vector.memset(ones64[64:65, :], 1.0)
            qt = cpool.tile([128, S], bf16, tag="qt")
            ktc = [
                cpool.tile([128, 512], bf16, tag=f"kt{ch}", name=f"kt{ch}")
                for ch in range(N_CH)
            ]
            vc = [
                cpool.tile([128, 4 * VW], bf16, tag=f"va{ch}", name=f"va{ch}")
                for ch in range(N_CH)
            ]
            attn_sb = [cpool.tile([64, S], bf16, tag="attn0", name="attn0")]

            def rope(dst, src_ps, cos_ap, sin_ap):
                xb = rpool.tile([128, 512], bf16, tag="rope_x")
                nc.vector.tensor_copy(xb[:], src_ps[:])
                sh = rpool.tile([128, 512], bf16, tag="rope_sh")
                nc.vector.stream_shuffle(sh[:], xb[:], PAIRSWAP)
                nc.vector.tensor_mul(xb[:], xb[:], cos_ap)
                nc.vector.tensor_mul(sh[:], sh[:], sin_ap)
                nc.vector.tensor_add(dst, xb[:], sh[:])

            def attention_tile(slot, T):
                """Score/exp/PV matmuls for one (head-slot, 256-row q-tile).

                Software-pipelined: group g+1's score matmuls are issued
                before group g's PV matmuls so the PE never sits behind the
                exp. Normalization is issued later (see finalize_tile)."""
                ro = 64 * slot
                qslice = qt[ro:ro + 64, T * 256:T * 256 + 256]
                pv = psv.tile([65, 256], f32, tag="ps_pv")
                groups = [(pg, 2 if pg + 1 <= T else 1) for pg in range(0, T + 1, 2)]

                def issue_pv(pg, w, et):
                    for pi in range(w):
                        p = pg + pi
                        for j in range(2):
                            t = 2 * p + j
                            nc.tensor.matmul(
                                pv[:],
                                vc[t // 4][:, (t % 4) * VW + slot * 65:(t % 4) * VW + slot * 65 + 65],
                                et[:, (2 * pi + j) * 256:(2 * pi + j + 1) * 256],
                                start=(p == 0 and j == 0),
                                stop=(p == T and j == 1),
                            )

                prev = None
                for pg, w in groups:
                    sc = pspool.tile([128, 1024], f32, tag="ps")
                    for pi in range(w):
                        for j in range(2):
                            t = 2 * (pg + pi) + j
                            nc.tensor.matmul(
                                sc[:, (2 * pi + j) * 256:(2 * pi + j + 1) * 256],
                                ktc[t // 4][ro:ro + 64, (t % 4) * 128:(t % 4) * 128 + 128],
                                qslice,
                                start=True,
                                stop=True,
                            )
                    et = epool.tile([128, 1024], bf16, tag="et")
                    nc.scalar.activation(
                        et[:, 0:512 * w], sc[:, 0:512 * w], Exp, bias=0.0, scale=0.125
                    )
                    if pg + w - 1 == T:  # group holds the diagonal pair
                        off = 512 * (w - 1)
                        nc.vector.tensor_mul(
                            et[:, off:off + 512], et[:, off:off + 512], maskab[:]
                        )
                    if prev is not None:
                        issue_pv(*prev)
                    prev = (pg, w, et)
                issue_pv(*prev)
                # reduce + reciprocal now; broadcast and normalize deferred
                pvs = npool.tile([65, 256], f32, tag="pvs")
                nc.vector.tensor_copy(pvs[:], pv[:])
                rrow = npool.tile([65, 256], bf16, tag="rrow")
                with nc.allow_low_precision(reason="bf16 softmax denominators"):
                    nc.vector.reciprocal(rrow[64:65, :], pvs[64:65, :])
                return pvs, rrow

            def b_tile(m):
                """Head-B attention for the core's m-th q-tile (physical
                q-tile 2m+parity, supplied pre-permuted in qb). Runs 2m+2
                pairs; the parity-dependent [128,1024] maskh input handles
                the diagonal and the padded pair in the last group."""
                qsl = qb[64:128, m * 256:m * 256 + 256]
                pv = psv.tile([65, 256], f32, tag="ps_pv")
                npairs = 2 * m + 2

                def issue_pvb(pg, et):
                    for pi in range(2):
                        p = pg + pi
                        for j in range(2):
                            t = 2 * p + j
                            nc.tensor.matmul(
                                pv[:],
                                vc[t // 4][:, (t % 4) * VW + 65:(t % 4) * VW + 130],
                                et[:, (2 * pi + j) * 256:(2 * pi + j + 1) * 256],
                                start=(p == 0 and j == 0),
                                stop=(p == npairs - 1 and j == 1),
                            )

                prev = None
                for pg in range(0, npairs, 2):
                    sc = pspool.tile([128, 1024], f32, tag="ps")
                    for pi in range(2):
                        for j in range(2):
                            t = 2 * (pg + pi) + j
                            nc.tensor.matmul(
                                sc[:, (2 * pi + j) * 256:(2 * pi + j + 1) * 256],
                                ktc[t // 4][64:128, (t % 4) * 128:(t % 4) * 128 + 128],
                                qsl,
                                start=True,
                                stop=True,
                            )
                    et = epool.tile([128, 1024], bf16, tag="et")
                    nc.scalar.activation(
                        et[:], sc[:], Exp, bias=0.0, scale=0.125
                    )
                    if pg + 2 >= npairs:  # last group: diagonal + padding
                        nc.vector.tensor_mul(et[:], et[:], maskh[:])
                    if prev is not None:
                        issue_pvb(*prev)
                    prev = (pg, et)
                issue_pvb(*prev)
                pvs = npool.tile([65, 256], f32, tag="pvs")
                nc.vector.tensor_copy(pvs[:], pv[:])
                rrow = npool.tile([65, 256], bf16, tag="rrow")
                with nc.allow_low_precision(reason="bf16 softmax denominators"):
                    nc.vector.reciprocal(rrow[64:65, :], pvs[64:65, :])
                return pvs, rrow

            def normalize(pvs, rrow, dst):
                rb = psv.tile([65, 256], f32, tag="ps_pv")
                nc.tensor.matmul(
                    rb[0:64, :], ones64[64:65, :], rrow[64:65, :],
                    start=True, stop=True,
                )
                nc.vector.tensor_mul(dst, pvs[0:64, :], rb[0:64, :])

            def finalize_tile(T, handles):
                """Deferred normalize (broadcast via K=1 matmul) + output
                projection for q-tile T; issued one tile later so the
                reciprocal is ready and the PE never waits."""
                pvs, rrow = handles
                normalize(pvs, rrow, attn_sb[0][0:64, T * 256:T * 256 + 256])
                o_proj_pair(T)

            def finalize_btile(m, handles):
                pvs, rrow = handles
                normalize(pvs, rrow, attnb[0:64, m * 256:m * 256 + 256])

            def o_proj_pair(T):
                osb = rbpool.tile([128, 2 * D_MODEL], bf16, tag="osb")
                for half in range(2):
                    qtl = 2 * T + half
                    pot = pspool.tile([128, 1024], f32, tag="ps")
                    po = pot[:, 0:512]
                    po2 = pot[:, 512:768]
                    lhsT = attn_sb[0][0:64, qtl * 128:qtl * 128 + 128]
                    # head-B contribution rides the same accumulation: the
                    # weight slice is real or zero depending on whether this
                    # core's parity owns q-tile T (selected by host data)
                    wb = D_MODEL + (T % 2) * D_MODEL
                    lhsB = attnb[0:64, (T // 2) * 256 + (qtl % 2) * 128:(T // 2) * 256 + (qtl % 2) * 128 + 128]
                    nc.tensor.matmul(po, lhsT, wo_sb[:, 0:512], start=True, stop=False)
                    nc.tensor.matmul(
                        po, lhsB, wo_sb[:, wb:wb + 512], start=False, stop=True
                    )
                    nc.tensor.matmul(
                        po2, lhsT, wo_sb[:, 512:768], start=True, stop=False
                    )
                    nc.tensor.matmul(
                        po2, lhsB, wo_sb[:, wb + 512:wb + 768], start=False, stop=True
                    )
                    nc.vector.tensor_copy(
                        osb[:, half * D_MODEL:(half + 1) * D_MODEL],
                        pot[:, 0:768],
                    )
                last_opart[0] = nc.sync.dma_start(
                    out=o_part[T * 256:(T + 1) * 256, :].rearrange(
                        "(h p) d -> p h d", p=128
                    ),
                    in_=osb[:].rearrange("p (h d) -> p h d", d=D_MODEL),
                )

            def fire_rs(j):
                _, lo, hi = RS_CHUNKS[j]
                nc.gpsimd.collective_compute(
                    "ReduceScatter",
                    mybir.AluOpType.add,
                    replica_groups=[list(range(N_CORES))],
                    ins=[o_part[lo:hi, :]],
                    outs=[ors[j][:]],
                )

            def readback(j):
                _, lo, hi = RS_CHUNKS[j]
                shard = (hi - lo) // 8
                for b in range(0, shard, 128):
                    bb = min(128, shard - b)
                    rt = rbpool.tile([128, D_MODEL], bf16, tag="rt")
                    rd = nc.sync.dma_start(out=rt[0:bb, :], in_=ors[j][b:b + bb, :])
                    # keep readback DMAs after all o_part writes so the
                    # round-robin DMA-queue counts of collective waits never
                    # include collective-dependent transfers
                    if last_opart[0] is not None:
                        add_dep_helper(
                            rd.ins, last_opart[0].ins, sync=True,
                            reason="readback after o_part stream",
                        )
                    rtf = rbpool.tile([128, D_MODEL], f32, tag="rtf")
                    nc.vector.tensor_copy(rtf[0:bb, :], rt[0:bb, :])
                    nc.sync.dma_start(
                        out=out_d[RS_OUT_OFF[j] + b:RS_OUT_OFF[j] + b + bb, :],
                        in_=rtf[0:bb, :],
                    )

            # ---- main loop: projection chunk ch, then attention q-tiles;
            # tile finalization (normalize + o_proj + RS) runs one tile behind
            rs_next = 0
            last_opart = [None]
            b_done = False
            pending = None  # (kind, idx, handles)

            def do_proj_kq(ch):
                def xt_t_slice(eb, lo, hi):
                    return xt_sb[:, eb * S + ch * 512 + lo:eb * S + ch * 512 + hi]
                ck = cosk_sb[:, ch * 512:(ch + 1) * 512]
                sk = sink_sb[:, ch * 512:(ch + 1) * 512]

                # K^T and Q^T share one PSUM tile; RoPE applied to both
                psKQ = pspool.tile([128, 1024], f32, tag="ps")
                for eb in range(EB):
                    nc.tensor.matmul(
                        psKQ[:, 0:512],
                        wk_sb[:, eb * 128:(eb + 1) * 128],
                        xt_t_slice(eb, 0, 512),
                        start=(eb == 0),
                        stop=(eb == EB - 1),
                    )
                for eb in range(EB):
                    nc.tensor.matmul(
                        psKQ[:, 512:1024],
                        wq_sb[:, eb * 128:(eb + 1) * 128],
                        xt_t_slice(eb, 0, 512),
                        start=(eb == 0),
                        stop=(eb == EB - 1),
                    )
                rope(ktc[ch][:], psKQ[:, 0:512], ck, sk)
                rope(qt[:, ch * 512:(ch + 1) * 512], psKQ[:, 512:1024], ck, sk)

            def do_proj_v(ch):
                def xt_t_slice(eb, lo, hi):
                    return xt_sb[:, eb * S + ch * 512 + lo:eb * S + ch * 512 + hi]
                # V (natural layout, interleaved ones column per head):
                # 4 s-tiles accumulate into one PSUM tile
                psV4 = pspool.tile([128, 1024], f32, tag="ps")
                for stl in range(4):
                    for eb in range(EB):
                        nc.tensor.matmul(
                            psV4[:, stl * 256:stl * 256 + 128],
                            xt_t_slice(eb, stl * 128, stl * 128 + 128),
                            wv_sb[:, eb * 128:(eb + 1) * 128],
                            start=(eb == 0),
                            stop=(eb == EB - 1),
                        )
                for stl in range(4):
                    vtile = vc[ch][:, stl * VW:(stl + 1) * VW].rearrange(
                        "p (h d) -> p h d", d=65
                    )
                    nc.vector.memset(vtile[:, :, 64:65], 1.0)
                    nc.vector.tensor_copy(
                        vtile[:, :, 0:64],
                        psV4[:, stl * 256:stl * 256 + 128].rearrange(
                            "p (h d) -> p h d", d=64
                        ),
                    )

            def do_proj_qb(q2):
                """Project + RoPE a 512-col quarter of the permuted head-B
                q-columns into qb rows 64:128. The RoPE pair-swap comes from
                a second projection against host-swapped weights, so every
                vector op runs full-width and no partition shuffle is
                needed (rows 0:64 carry unused garbage)."""
                qs = slice(q2 * 512, q2 * 512 + 512)
                psB = pspool.tile([128, 1024], f32, tag="ps")
                for eb in range(EB):
                    nc.tensor.matmul(
                        psB[64:128, 0:512],
                        wq_sb[:, eb * 128 + 64:(eb + 1) * 128],
                        xqb_sb[:, eb * 2048 + q2 * 512:eb * 2048 + q2 * 512 + 512],
                        start=(eb == 0),
                        stop=(eb == EB - 1),
                    )
                for eb in range(EB):
                    nc.tensor.matmul(
                        psB[64:128, 512:1024],
                        wqbs_sb[:, eb * 64:(eb + 1) * 64],
                        xqb_sb[:, eb * 2048 + q2 * 512:eb * 2048 + q2 * 512 + 512],
                        start=(eb == 0),
                        stop=(eb == EB - 1),
                    )
                xb = rpool.tile([128, 512], bf16, tag="rope_x")
                nc.vector.tensor_copy(xb[64:128, :], psB[64:128, 0:512])
                sh = rpool.tile([128, 512], bf16, tag="rope_sh")
                nc.vector.tensor_copy(sh[64:128, :], psB[64:128, 512:1024])
                nc.vector.tensor_mul(xb[64:128, :], xb[64:128, :], cosqb_sb[64:128, qs])
                nc.vector.tensor_mul(sh[64:128, :], sh[64:128, :], sinqb_sb[64:128, qs])
                nc.vector.tensor_add(qb[64:128, qs], xb[64:128, :], sh[64:128, :])

            # projections run one chunk ahead, split and issued mid-chunk so
            # the attention tiles keep the Act engine fed at boundaries
            do_proj_kq(0)
            do_proj_v(0)
            do_proj_qb(0)
            do_proj_kq(1)
            do_proj_v(1)

            def pop_pending():
                nonlocal pending, rs_next, b_done
                if pending is None:
                    return
                kind, idx, ph = pending
                if kind == "A":
                    finalize_tile(idx, ph)
                    if (
                        with_rs
                        and rs_next < len(RS_CHUNKS)
                        and RS_CHUNKS[rs_next][0] == idx
                    ):
                        fire_rs(rs_next)
                        rs_next += 1
                else:
                    finalize_btile(idx, ph)
                pending = None

            for ch in range(N_CH):
                bh = b_tile(ch)
                pop_pending()
                pending = ("B", ch, bh)
                for T in (2 * ch, 2 * ch + 1):
                    handles = attention_tile(0, T)
                    pop_pending()
                    pending = ("A", T, handles)
                    if ch + 2 < N_CH:
                        if T == 2 * ch:
                            do_proj_kq(ch + 2)
                        else:
                            do_proj_v(ch + 2)
                    if ch in (1, 3, 5) and T == 2 * ch:
                        do_proj_qb((ch + 1) // 2)
            pop_pending()
            while with_rs and rs_next < len(RS_CHUNKS):
                fire_rs(rs_next)
                rs_next += 1
            if with_rs:
                for j in range(len(RS_CHUNKS)):
                    readback(j)


    nc.compile()
    return nc


_PROGRAM = None


def _get_program():
    global _PROGRAM
    if _PROGRAM is None:
        _PROGRAM = build_program()
    return _PROGRAM


def host_prep(in_features, token_positions, q_proj, k_proj, v_proj, o_proj):
    """Build the 8 per-core input maps."""
    x = np.asarray(in_features, np.float32).reshape(S, D_MODEL)
    tp = np.asarray(token_positions)
    qp = np.asarray(q_proj, np.float32)
    kp = np.asarray(k_proj, np.float32)
    vp = np.asarray(v_proj, np.float32)
    op = np.asarray(o_proj, np.float32)

    xt_bf = np.ascontiguousarray(x.T).astype(BF16)      # [768, 4096]
    wqT = np.ascontiguousarray(qp.T)                    # [in 768, out 768]
    wkT = np.ascontiguousarray(kp.T)
    wvT = np.ascontiguousarray(vp.T)
    opT = np.ascontiguousarray(op.T)                    # [in-dk 768, out 768]

    inv_freq = 1.0 / THETA ** (np.arange(0, DK, 2, dtype=np.float32) / DK)
    pos = np.clip(tp.astype(np.float32), 0, MAX_SEQ_LEN - 1)
    freq = pos[:, None] * inv_freq[None, :]             # [S, 32]
    cos_t, sin_t = np.cos(freq), np.sin(freq)

    r = np.arange(128)
    fidx = (r % 64) // 2
    sign = np.where(r % 2 == 0, -1.0, 1.0).astype(np.float32)
    cos128 = cos_t[:, fidx].T.astype(BF16)              # [128, S]
    sin128 = (sin_t[:, fidx].T * sign[:, None]).astype(BF16)

    ki = np.arange(128)[:, None]
    qi = np.arange(256)[None, :]
    mask_a = (ki <= qi).astype(np.float32)
    mask_b = (ki + 128 <= qi).astype(np.float32)
    maskab = np.concatenate([mask_a, mask_b], axis=1).astype(BF16)

    ones512 = np.ones((128, 512), np.float32)
    zeros512 = np.zeros((128, 512), np.float32)

    in_maps = []
    for c in range(N_CORES):
        hA = c
        hB = 8 + c // 2
        p = c % 2

        def wslice(wT):
            out = np.empty((D_MODEL, 128), np.float32)
            out[:, 0:64] = wT[:, hA * 64:(hA + 1) * 64]
            out[:, 64:128] = wT[:, hB * 64:(hB + 1) * 64]
            return out.astype(BF16)

        wo2 = np.zeros((64, 3 * D_MODEL), np.float32)
        wo2[:, 0:D_MODEL] = opT[hA * 64:(hA + 1) * 64, :]
        # slot 1 = used for even q-tiles, slot 2 = odd q-tiles; only the
        # parity this core owns carries real head-B weights
        wo2[:, (1 + p) * D_MODEL:(2 + p) * D_MODEL] = opT[hB * 64:(hB + 1) * 64, :]

        wqB = wqT[:, hB * 64:(hB + 1) * 64]
        swap = np.arange(64) ^ 1
        wqbs = np.ascontiguousarray(wqB[:, swap]).astype(BF16)

        qcolsB = np.concatenate(
            [np.arange(256 * (2 * m + p), 256 * (2 * m + p) + 256) for m in range(8)]
        )
        maskab_f = maskab.astype(np.float32)
        if p == 0:
            maskh = np.concatenate([maskab_f, zeros512], axis=1)
        else:
            maskh = np.concatenate([ones512, maskab_f], axis=1)

        in_maps.append(
            {
                "xt": xt_bf,
                "xqb": np.ascontiguousarray(xt_bf[:, qcolsB]),
                "wqbs": wqbs,
                "wq2": wslice(wqT),
                "wk2": wslice(wkT),
                "wv2": wslice(wvT),
                "wo2": wo2.astype(BF16),
                "cosk": cos128,
                "sink": sin128,
                "cosqb": np.ascontiguousarray(cos128[0:64, qcolsB]),
                "sinqb": np.ascontiguousarray(sin128[0:64, qcolsB]),
                "maskab": maskab,
                "maskh": maskh.astype(BF16),
            }
        )
    return in_maps


def assemble_output(results):
    out = np.empty((1, S, D_MODEL), np.float32)
    for c in range(N_CORES):
        r = np.asarray(results[c]["out"], np.float32)
        for j, (_, lo, hi) in enumerate(RS_CHUNKS):
            shard = (hi - lo) // 8
            oo = RS_OUT_OFF[j]
            out[0, lo + shard * c:lo + shard * (c + 1)] = r[oo:oo + shard]
    return out


def kernel(**inputs):
    from concourse.bass_utils import run_bass_kernel_spmd

    nc = _get_program()
    in_maps = host_prep(**inputs)
    res = run_bass_kernel_spmd(nc, in_maps, list(range(N_CORES)))
    return assemble_output(res.results)


if __name__ == "__main__":
    nc = build_program()
    print("program built and compiled")
